# revision 1
# baseline (speedup 1.0000x reference)
"""GAT (2-layer, PyG GATConv-style) on 8 Trainium2 NeuronCores.

Strategy (dst-partitioned message passing):
  - Nodes are split into 8 shards of 6250 (padded to 6272 = 49*128 per core).
  - Edges (incl. self-loops) are sorted by dst on the host and routed to the
    core owning the dst; within a core they are grouped into 49 windows of
    128 dst nodes, each window padded to a fixed number of 128-edge blocks
    (lo/hi split so gather indices fit int16 for dma_gather).
  - Per layer: every core computes its shard of the node feature table
    (h = x @ W plus folded attention scores), cores AllGather the table,
    then each core dma_gathers source rows per edge, forms
    p = exp(leakyrelu(al_src+al_dst+al_edge)) (softmax max-shift skipped;
    logits are O(1) so exp is safe), scales messages by p and scatter-adds
    them per dst window via a one-hot matmul on the tensor engine. The
    softmax denominator rides along as extra matmul columns; normalization
    happens after the scatter.
  - Graph mean-pool is a batch-one-hot matmul per core; host sums the 8
    partial [64, 65] (sums|counts) outputs and divides.

Only index bookkeeping (sort/permutation/padding) and small weight folding
is done on the host; all O(N*F)/O(E*F) floating point math runs on device.
"""

import sys

for _p in ("/opt/trn_rl_repo",):
    if _p not in sys.path:
        sys.path.insert(0, _p)

from contextlib import ExitStack

import numpy as np

import concourse.bass as bass
import concourse.mybir as mybir
import concourse.tile as tile
from concourse import bacc
from concourse.bass_utils import run_bass_kernel_spmd

F32 = mybir.dt.float32
I16 = mybir.dt.int16
AF = mybir.ActivationFunctionType
OP = mybir.AluOpType

NCORES = 8
N, E, FIN, ED = 50000, 400000, 128, 6
H, C1, C2, G = 4, 64, 64, 64
EPS = 1e-5
P = 128
SH = N // NCORES            # 6250 nodes per shard
WPC = (SH + P - 1) // P     # 49 windows per core
PADN = WPC * P              # 6272 padded shard rows
HALF = 4 * PADN             # 25088 rows per half-table (int16-safe)
R1, U1 = 320, 260           # L1 table row f32 elems (1280B), used prefix
R2, U2 = 128, 65            # L2 table row f32 elems (512B), used prefix
RD = 64                     # al_dst table row (256B)
NEG = -1.0e9


def _wrap16(vals):
    """[..., L] int -> [..., 128, L//16] int16, idx j at (j%16, j//16),
    replicated across the 8 gpsimd core windows."""
    lead = vals.shape[:-1]
    L = vals.shape[-1]
    out = np.zeros(lead + (16, L // 16), np.int16)
    jj = np.arange(L)
    out[..., jj % 16, jj // 16] = vals.astype(np.int16)
    return np.tile(out, lead and (1, 8, 1) or (8, 1))


# ----------------------------------------------------------------- host prep
def _prep(inputs):
    x = np.asarray(inputs["x"], np.float32)
    ei = np.asarray(inputs["edge_index"])
    ea = np.asarray(inputs["edge_attr"], np.float32)
    batch = np.asarray(inputs["batch"])
    W1 = np.asarray(inputs["W1"], np.float32)
    We1 = np.asarray(inputs["We1"], np.float32)
    a_src1 = np.asarray(inputs["a_src1"], np.float32)
    a_dst1 = np.asarray(inputs["a_dst1"], np.float32)
    a_edge1 = np.asarray(inputs["a_edge1"], np.float32)
    b1 = np.asarray(inputs["b1"], np.float32)
    ln1_w = np.asarray(inputs["ln1_w"], np.float32)
    ln1_b = np.asarray(inputs["ln1_b"], np.float32)
    W2 = np.asarray(inputs["W2"], np.float32)
    We2 = np.asarray(inputs["We2"], np.float32)
    a_src2 = np.asarray(inputs["a_src2"], np.float32)
    a_dst2 = np.asarray(inputs["a_dst2"], np.float32)
    a_edge2 = np.asarray(inputs["a_edge2"], np.float32)
    b2 = np.asarray(inputs["b2"], np.float32)
    ln2_w = np.asarray(inputs["ln2_w"], np.float32)
    ln2_b = np.asarray(inputs["ln2_b"], np.float32)

    # edges + self loops, sorted by dst
    loop = np.arange(N, dtype=np.int64)
    src = np.concatenate([ei[0].astype(np.int64), loop])
    dst = np.concatenate([ei[1].astype(np.int64), loop])
    order = np.argsort(dst, kind="stable")
    src, dst = src[order], dst[order]
    ea_mean = ea.mean(0)
    ea_sorted = np.empty((len(src), ED), np.float32)
    is_loop = order >= E
    ea_sorted[~is_loop] = ea[order[~is_loop]]
    ea_sorted[is_loop] = ea_mean

    src_row = (src // SH) * PADN + src % SH      # row in the full table
    is_lo = src_row < HALF

    # per (core, window) lo/hi counts -> global fixed block counts
    core_of = dst // SH
    win_of = (dst - core_of * SH) // P
    gwin = core_of * WPC + win_of
    nlo = np.bincount(gwin[is_lo], minlength=NCORES * WPC)
    nhi = np.bincount(gwin[~is_lo], minlength=NCORES * WPC)
    SLB = int(np.ceil(nlo.max() / P))
    SHB = int(np.ceil(nhi.max() / P))
    nbk = SLB + SHB
    slots = nbk * P

    counts = np.bincount(gwin, minlength=NCORES * WPC)
    starts = np.zeros(NCORES * WPC + 1, np.int64)
    np.cumsum(counts, out=starts[1:])

    # folded attention weight vectors
    u1s = (W1.reshape(FIN, H, C1) * a_src1[None]).sum(-1)
    u1d = (W1.reshape(FIN, H, C1) * a_dst1[None]).sum(-1)
    v1 = (We1.reshape(ED, H, C1) * a_edge1[None]).sum(-1)
    u2s = (W2.reshape(H * C1, 1, C2) * a_src2[None]).sum(-1)
    u2d = (W2.reshape(H * C1, 1, C2) * a_dst2[None]).sum(-1)
    v2 = (We2.reshape(ED, 1, C2) * a_edge2[None]).sum(-1)

    w1cat = np.concatenate([W1, u1s, u1d], 1)                 # [128, 264]
    w2cat = np.concatenate([W2, u2s, u2d], 1)                 # [256, 66]
    v1p = np.concatenate([v1, np.full((1, H), NEG, np.float32)], 0)
    v2p = np.concatenate([v2, np.full((1, 1), NEG, np.float32)], 0)

    rep = lambda v: np.broadcast_to(v[None, :], (P, v.shape[0])).copy()
    iota128 = np.broadcast_to(
        np.arange(P, dtype=np.float32)[None, :], (P, P)).copy()
    iota64 = np.broadcast_to(
        np.arange(G, dtype=np.float32)[None, :], (P, G)).copy()

    shared = dict(
        w1cat=np.ascontiguousarray(w1cat, np.float32),
        w2cat=np.ascontiguousarray(w2cat, np.float32),
        v1p=v1p.astype(np.float32),
        v2p=v2p.astype(np.float32),
        b1r=rep(b1), ln1wr=rep(ln1_w), ln1br=rep(ln1_b),
        b2r=rep(b2), ln2wr=rep(ln2_w), ln2br=rep(ln2_b),
        iota128=iota128, iota64=iota64,
    )

    in_maps = []
    for c in range(NCORES):
        lo = c * SH
        xT = np.zeros((FIN, PADN), np.float32)
        xT[:, :SH] = x[lo:lo + SH].T
        glo = np.zeros((WPC, SLB * P), np.int64)
        ghi = np.zeros((WPC, SHB * P), np.int64)
        dsti = np.zeros((WPC, slots), np.int64)
        dcol = np.zeros((WPC, P, nbk), np.float32)
        eaT = np.zeros((WPC, ED + 1, slots), np.float32)
        eaT[:, ED, :] = 1.0  # pad flag on by default
        bcol = np.full((WPC, P), 999.0, np.float32)
        bcol.reshape(-1)[:SH] = batch[lo:lo + SH].astype(np.float32)

        for w in range(WPC):
            g = c * WPC + w
            s, e = starts[g], starts[g + 1]
            if e == s:
                continue
            sr = src_row[s:e]
            ed_ = dst[s:e]
            eaw = ea_sorted[s:e]
            ml = sr < HALF
            for half, msel, base, tab in (
                (0, ml, 0, glo), (1, ~msel if False else ~ml, SLB, ghi),
            ):
                idxs = np.nonzero(msel)[0]
                n_h = len(idxs)
                if n_h == 0:
                    continue
                jj = np.arange(n_h)
                pp, kk = jj % P, base + jj // P
                slot = kk * P + pp
                tab[w, jj] = sr[idxs] - (HALF if half else 0)
                dsti[w, slot] = ed_[idxs] - lo
                dcol[w, pp, kk] = (ed_[idxs] - lo - w * P).astype(np.float32)
                eaT[w, :ED, slot] = eaw[idxs]  # adv-index puts slot axis first
                eaT[w, ED, slot] = 0.0

        m = dict(shared)
        m.update(
            xT=xT,
            glo=_wrap16(glo), ghi=_wrap16(ghi), dsti=_wrap16(dsti),
            dcol=dcol, eaT=eaT, bcol=bcol,
        )
        in_maps.append(m)
    return in_maps, (SLB, SHB)


# ------------------------------------------------------------- device program
def _build(blocks):
    SLB, SHB = blocks
    nbk = SLB + SHB
    slots = nbk * P
    nc = bacc.Bacc("TRN2", target_bir_lowering=False, debug=False,
                   num_devices=NCORES)
    rg = [list(range(NCORES))]

    t_in = {}
    for name, shape, dt in [
        ("xT", [FIN, PADN], F32),
        ("w1cat", [FIN, 264], F32),
        ("w2cat", [H * C1, C2 + 2], F32),
        ("v1p", [ED + 1, H], F32),
        ("v2p", [ED + 1, 1], F32),
        ("b1r", [P, H * C1], F32), ("ln1wr", [P, H * C1], F32),
        ("ln1br", [P, H * C1], F32),
        ("b2r", [P, C2], F32), ("ln2wr", [P, C2], F32),
        ("ln2br", [P, C2], F32),
        ("iota128", [P, P], F32), ("iota64", [P, G], F32),
        ("glo", [WPC, P, SLB * P // 16], I16),
        ("ghi", [WPC, P, SHB * P // 16], I16),
        ("dsti", [WPC, P, slots // 16], I16),
        ("dcol", [WPC, P, nbk], F32),
        ("eaT", [WPC, ED + 1, slots], F32),
        ("bcol", [WPC, P], F32),
    ]:
        t_in[name] = nc.dram_tensor(name, shape, dt, kind="ExternalInput")
    out_partial = nc.dram_tensor("partial", [G, G + 1], F32,
                                 kind="ExternalOutput")

    with tile.TileContext(nc) as tc, ExitStack() as ctx:
        const = ctx.enter_context(tc.tile_pool(name="const", bufs=1))
        work = ctx.enter_context(tc.tile_pool(name="work", bufs=3))
        big = ctx.enter_context(tc.tile_pool(name="big", bufs=1))
        psum = ctx.enter_context(tc.tile_pool(name="psum", bufs=2,
                                              space="PSUM"))
        dram = ctx.enter_context(tc.tile_pool(name="dram", bufs=1,
                                              space="DRAM"))

        # const APs used as activation bias operands
        zero_t = const.tile([P, 1], F32)
        nc.vector.memset(zero_t[:], 0.0)
        nc.const_aps.aps[(F32, 0.0)] = zero_t[:]
        eps_t = const.tile([P, 1], F32)
        nc.vector.memset(eps_t[:], EPS)
        nc.const_aps.aps[(F32, EPS)] = eps_t[:]

        def cload(name):
            src_t = t_in[name]
            t = const.tile(list(src_t.shape), F32, name=f"c_{name}")
            nc.sync.dma_start(t[:], src_t.ap())
            return t

        w1cat_sb = cload("w1cat")
        v1p_sb = cload("v1p")
        v2p_sb = cload("v2p")
        b1_sb = cload("b1r"); ln1w_sb = cload("ln1wr"); ln1b_sb = cload("ln1br")
        b2_sb = cload("b2r"); ln2w_sb = cload("ln2wr"); ln2b_sb = cload("ln2br")
        iota128_sb = cload("iota128")
        iota64_sb = cload("iota64")
        w2a_sb = const.tile([P, C2 + 2], F32)
        nc.sync.dma_start(w2a_sb[:], t_in["w2cat"].ap()[0:P, :])
        w2b_sb = const.tile([P, C2 + 2], F32)
        nc.sync.dma_start(w2b_sb[:], t_in["w2cat"].ap()[P:2 * P, :])
        from concourse.masks import make_identity
        ident_sb = const.tile([P, P], F32)
        make_identity(nc, ident_sb[:])

        # DRAM scratch
        bounce1 = dram.tile([PADN, R1], F32)
        table1 = dram.tile([NCORES * PADN, R1], F32, addr_space="Shared")
        ald1_tab = dram.tile([PADN, RD], F32)
        bounce2 = dram.tile([PADN, R2], F32)
        table2 = dram.tile([NCORES * PADN, R2], F32, addr_space="Shared")
        ald2_tab = dram.tile([PADN, RD], F32)

        accum1 = big.tile([P, WPC, U1], F32)
        h3_sb = big.tile([P, WPC, G + 1], F32)

        # ---------------- phase 0: h_aug = x @ [W1|u1s|u1d] -> table1
        for w in range(WPC):
            xt0 = work.tile([FIN, P], F32, tag="xt0")
            nc.sync.dma_start(xt0[:], t_in["xT"].ap()[:, w * P:(w + 1) * P])
            ps0 = psum.tile([P, 264], F32, tag="mm", bufs=3)
            nc.tensor.matmul(ps0[:], lhsT=xt0[:], rhs=w1cat_sb[:],
                             start=True, stop=True)
            st0 = work.tile([P, 264], F32, tag="st0")
            nc.scalar.activation(st0[:], ps0[:], AF.Copy)
            nc.sync.dma_start(bounce1[w * P:(w + 1) * P, 0:U1], st0[:, 0:U1])
            nc.sync.dma_start(ald1_tab[w * P:(w + 1) * P, 0:H],
                              st0[:, U1:U1 + H])
        # AllGather writes shard rows [c*PADN:(c+1)*PADN] of table1
        nc.gpsimd.collective_compute(
            "AllGather", OP.bypass, replica_groups=rg,
            ins=[bounce1.opt()], outs=[table1.opt()])

        # ---------------- edge phase (both layers)
        ilo_all = big.tile([P, WPC, SLB * P // 16], I16)
        nc.sync.dma_start(ilo_all[:],
                          t_in["glo"].ap().rearrange("w p c -> p w c"))
        ihi_all = big.tile([P, WPC, SHB * P // 16], I16)
        nc.sync.dma_start(ihi_all[:],
                          t_in["ghi"].ap().rearrange("w p c -> p w c"))
        idt_all = big.tile([P, WPC, slots // 16], I16)
        nc.sync.dma_start(idt_all[:],
                          t_in["dsti"].ap().rearrange("w p c -> p w c"))
        dc_all = big.tile([P, WPC, SLB + SHB], F32)
        nc.sync.dma_start(dc_all[:],
                          t_in["dcol"].ap().rearrange("w p k -> p w k"))

        def edge_phase(table, rowlen, used, nh, ald_tab, vp_sb, accum, tagp):
            for w in range(WPC):
                ilo = ilo_all[:, w]
                ihi = ihi_all[:, w]
                idt = idt_all[:, w]
                gb = work.tile([P, nbk, rowlen], F32, tag=f"gb{tagp}",
                               bufs=2)
                nc.gpsimd.dma_gather(
                    out_ap=gb[:, 0:SLB, :], in_ap=table[0:HALF, :],
                    idxs_ap=ilo, num_idxs=SLB * P, num_idxs_reg=SLB * P,
                    elem_size=rowlen, single_packet=False)
                nc.gpsimd.dma_gather(
                    out_ap=gb[:, SLB:nbk, :], in_ap=table[HALF:2 * HALF, :],
                    idxs_ap=ihi, num_idxs=SHB * P, num_idxs_reg=SHB * P,
                    elem_size=rowlen, single_packet=False)
                ag = work.tile([P, nbk, RD], F32, tag="ag", bufs=3)
                nc.gpsimd.dma_gather(
                    out_ap=ag[:], in_ap=ald_tab[:], idxs_ap=idt,
                    num_idxs=slots, num_idxs_reg=slots,
                    elem_size=RD, single_packet=False)
                eat = work.tile([ED + 1, slots], F32, tag="eat")
                nc.sync.dma_start(eat[:], t_in["eaT"].ap()[w])
                dc = dc_all[:, w]

                # one-hot [e_p, k, j]
                oh = work.tile([P, nbk, P], F32, tag="oh", bufs=3)
                nc.vector.tensor_tensor(
                    out=oh[:],
                    in0=iota128_sb[:].rearrange("p (o j) -> p o j", o=1)
                        .to_broadcast([P, nbk, P]),
                    in1=dc.to_broadcast([P, nbk, P]),
                    op=OP.is_equal)

                # al_edge via PE: [7,128] x [7,nh] -> psum per block
                zps = psum.tile([P, nbk * nh], F32, tag="zps", bufs=2)
                for k in range(nbk):
                    nc.tensor.matmul(
                        zps[:, k * nh:(k + 1) * nh],
                        lhsT=eat[:, k * P:(k + 1) * P],
                        rhs=vp_sb[:], start=True, stop=True)

                # z = als_g + ald_g + ale ; p = exp(max(z, 0.2 z))
                z = work.tile([P, nbk * nh], F32, tag="z")
                nc.vector.tensor_add(
                    z[:].rearrange("p (k h) -> p k h", h=nh),
                    gb[:, :, used - nh:used], ag[:, :, 0:nh])
                nc.vector.tensor_add(z[:], z[:], zps[:])
                nc.vector.scalar_tensor_tensor(
                    out=z[:], in0=z[:], scalar=0.2, in1=z[:],
                    op0=OP.mult, op1=OP.max)
                nc.scalar.activation(z[:], z[:], AF.Exp)

                # p into the al_src cols, msg *= p
                nc.vector.tensor_copy(
                    gb[:, :, used - nh:used],
                    z[:].rearrange("p (k h) -> p k h", h=nh))
                nc.vector.tensor_tensor(
                    out=gb[:, :, 0:used - nh]
                        .rearrange("p k (h c) -> p k h c", h=nh),
                    in0=gb[:, :, 0:used - nh]
                        .rearrange("p k (h c) -> p k h c", h=nh),
                    in1=z[:].rearrange("p (k h) -> p k h", h=nh)
                        .to_broadcast([P, nbk, nh, (used - nh) // nh]),
                    op=OP.mult)

                # scatter-add via one-hot matmul
                sc = psum.tile([P, used], F32, tag="sc", bufs=2)
                for k in range(nbk):
                    nc.tensor.matmul(
                        sc[:], lhsT=oh[:, k, :], rhs=gb[:, k, 0:used],
                        start=(k == 0), stop=(k == nbk - 1))
                nc.scalar.activation(accum[:, w, :], sc[:], AF.Copy)

        edge_phase(table1, R1, U1, H, ald1_tab, v1p_sb, accum1, "a")

        # ---------------- normalize + relu + LN (in place over accum)
        def norm_ln(accum, w, nh, feat, y_out, b_sb, lnw_sb, lnb_sb):
            acc = accum[:, w, :]
            rec = work.tile([P, nh], F32, tag="rec")
            nc.vector.tensor_scalar_add(rec[:], acc[:, feat:feat + nh], 1e-16)
            nc.vector.reciprocal(rec[:], rec[:])
            nc.vector.tensor_tensor(
                out=y_out.rearrange("p (h c) -> p h c", h=nh),
                in0=acc[:, 0:feat].rearrange("p (h c) -> p h c", h=nh),
                in1=rec[:].to_broadcast([P, nh, feat // nh]),
                op=OP.mult)
            nc.vector.tensor_add(y_out, y_out, b_sb[:, 0:feat])
            nc.scalar.activation(y_out, y_out, AF.Relu)
            mu = work.tile([P, 1], F32, tag="mu")
            nc.vector.tensor_reduce(mu[:], y_out, axis=mybir.AxisListType.X,
                                    op=OP.add)
            mus = work.tile([P, 1], F32, tag="mus")
            nc.scalar.activation(mus[:], mu[:], AF.Copy, scale=1.0 / feat)
            nc.vector.tensor_scalar_sub(y_out, y_out, mus[:, 0:1])
            sq = work.tile([P, feat], F32, tag="sq")
            var = work.tile([P, 1], F32, tag="var")
            nc.scalar.activation(sq[:], y_out, AF.Square, accum_out=var[:])
            sd = work.tile([P, 1], F32, tag="sd")
            nc.scalar.activation(sd[:], var[:], AF.Sqrt, bias=EPS,
                                 scale=1.0 / feat)
            nc.vector.reciprocal(sd[:], sd[:])
            nc.vector.tensor_scalar_mul(y_out, y_out, sd[:, 0:1])
            nc.vector.tensor_tensor(y_out, y_out, lnw_sb[:, 0:feat], OP.mult)
            nc.vector.tensor_add(y_out, y_out, lnb_sb[:, 0:feat])

        for w in range(WPC):
            norm_ln(accum1, w, H, H * C1, accum1[:, w, 0:H * C1], b1_sb,
                    ln1w_sb, ln1b_sb)

        # ---------------- layer-2 prep: [hp | als2 | ald2] = h2 @ w2cat
        for w in range(WPC):
            ps2 = psum.tile([P, C2 + 2], F32, tag="mm", bufs=3)
            for fb in range(2):
                tp = psum.tile([P, P], F32, tag="mm", bufs=3)
                nc.tensor.transpose(tp[:], accum1[:, w, fb * P:(fb + 1) * P],
                                    ident_sb[:])
                tsb = work.tile([P, P], F32, tag="tsb")
                nc.vector.tensor_copy(tsb[:], tp[:])
                nc.tensor.matmul(ps2[:], lhsT=tsb[:],
                                 rhs=(w2a_sb[:] if fb == 0 else w2b_sb[:]),
                                 start=(fb == 0), stop=(fb == 1))
            st2 = work.tile([P, C2 + 2], F32, tag="st2")
            nc.vector.tensor_copy(st2[:], ps2[:])
            nc.sync.dma_start(bounce2[w * P:(w + 1) * P, 0:U2], st2[:, 0:U2])
            nc.sync.dma_start(ald2_tab[w * P:(w + 1) * P, 0:1],
                              st2[:, C2 + 1:C2 + 2])
        nc.gpsimd.collective_compute(
            "AllGather", OP.bypass, replica_groups=rg,
            ins=[bounce2.opt()], outs=[table2.opt()])

        # ---------------- layer-2 edges + normalize
        accum2 = big.tile([P, WPC, U2], F32)
        edge_phase(table2, R2, U2, 1, ald2_tab, v2p_sb, accum2, "b")

        nc.vector.memset(h3_sb[:], 1.0)
        for w in range(WPC):
            norm_ln(accum2, w, 1, C2, h3_sb[:, w, 0:C2], b2_sb, ln2w_sb,
                    ln2b_sb)

        # ---------------- graph mean-pool partials
        pl = psum.tile([G, G + 1], F32, tag="pl", bufs=1)
        for w in range(WPC):
            bc = work.tile([P, 1], F32, tag="bc")
            nc.sync.dma_start(bc[:], t_in["bcol"].ap()[w, :, None])
            bh = work.tile([P, G], F32, tag="bh")
            nc.vector.tensor_scalar(
                out=bh[:], in0=iota64_sb[:], scalar1=bc[:, 0:1],
                scalar2=None, op0=OP.is_equal)
            nc.tensor.matmul(pl[:], lhsT=bh[:], rhs=h3_sb[:, w, 0:G + 1],
                             start=(w == 0), stop=(w == WPC - 1))
        plo = work.tile([G, G + 1], F32)
        nc.vector.tensor_copy(plo[:], pl[:])
        nc.sync.dma_start(out_partial.ap(), plo[:])

    nc.compile()
    return nc


_CACHE = {}


def _get_program(blocks):
    if blocks not in _CACHE:
        _CACHE[blocks] = _build(blocks)
    return _CACHE[blocks]


def _run(inputs, trace=False):
    in_maps, blocks = _prep(inputs)
    nc = _get_program(blocks)
    res = run_bass_kernel_spmd(nc, in_maps, core_ids=list(range(NCORES)),
                               trace=trace)
    total = np.zeros((G, G + 1), np.float64)
    for c in range(NCORES):
        total += res.results[c]["partial"].astype(np.float64)
    out = total[:, :G] / np.maximum(total[:, G:G + 1], 1.0)
    return out.astype(np.float32), res


def kernel(**inputs):
    out, _ = _run(inputs, trace=False)
    return out



# revision 13
# speedup vs baseline: 1.1758x; 1.1758x over previous
"""GAT (2-layer, PyG GATConv-style) on 8 Trainium2 NeuronCores.

Strategy (dst-partitioned message passing, memory-optimized):
  - Nodes split into 8 shards of 6250 (padded to 6272 = 49*128 per core).
  - Edges (incl. self-loops) sorted by dst, routed to the dst-owning core,
    grouped into 49 windows of 128 dst nodes, each window padded to fixed
    128-edge blocks (lo/hi split by src half so gather indices fit int16).
  - Layer 1 table rows are 512B: 256 fp8e4m3 h values + 4 bf16 al_src
    scores; the table is AllGathered (25.7MB vs 64MB in f32) and each core
    dma_gathers source rows per edge.
  - Layer 2 table rows are 256B and pack TWO nodes ([64 fp8 h + 1 bf16
    al_src] each); the consumer selects the half by parity, halving the
    AllGather to 6.4MB.
  - al_dst is applied via a transposed one-hot matmul on the tensor engine
    (al_dst of a window's 128 dst nodes stays in SBUF) instead of a
    256B-per-edge gather.
  - Edge-attr loads, al_edge matmuls and one-hot builds are issued between
    the AllGather and the table gathers so they execute during the
    collective.
  - Per-edge: p = exp(leakyrelu(al_src+al_dst+al_edge)) (max-shift skipped;
    logits are O(1)); messages scaled by p on DVE (scalar_tensor_tensor,
    all-SBUF 2x mode) and scatter-added per dst window via bf16 one-hot
    matmuls; softmax denominator rides as extra matmul columns.
  - LayerNorm rsqrt via Ln+Exp so every activation lives in one act table.
  - Graph mean-pool via batch-one-hot matmul; host sums 8 partial [64,65]
    outputs and divides.

Host does only index bookkeeping and small-weight folding; all O(N*F) /
O(E*F) floating-point math runs on device.
"""

import sys

for _p in ("/opt/trn_rl_repo",):
    if _p not in sys.path:
        sys.path.insert(0, _p)

from contextlib import ExitStack

import ml_dtypes
import numpy as np

import concourse.bass as bass
import concourse.mybir as mybir
import concourse.tile as tile
from concourse import bacc
from concourse.bass_utils import run_bass_kernel_spmd

F32 = mybir.dt.float32
BF16 = mybir.dt.bfloat16
F8 = mybir.dt.float8e4
U8 = mybir.dt.uint8
I16 = mybir.dt.int16
AF = mybir.ActivationFunctionType
OP = mybir.AluOpType
BF = ml_dtypes.bfloat16

NCORES = 8
N, E, FIN, ED = 50000, 400000, 128, 6
H, C1, C2, G = 4, 64, 64, 64
EPS = 1e-5
P = 128
SH = N // NCORES            # 6250 nodes per shard
WPC = (SH + P - 1) // P     # 49 windows per core
PADN = WPC * P              # 6272 padded shard rows
HALF = 4 * PADN             # 25088 rows per half-table (int16-safe)
HB2 = PADN // 2             # 3136 pair rows per core (layer-2 table)
ROW1 = 512                  # L1 table row bytes: 256 fp8 h + 4 bf16 al_src
ROW2 = 256                  # L2 table row bytes: 2x(64 fp8 h + 1 bf16 al)
NEG = -1.0e9
EGRP = 1                    # windows per eaT load


def _wrap16(vals):
    """[..., L] int -> [..., 128, L//16] int16, idx j at (j%16, j//16),
    replicated across the 8 gpsimd core windows."""
    lead = vals.shape[:-1]
    L = vals.shape[-1]
    out = np.zeros(lead + (16, L // 16), np.int16)
    jj = np.arange(L)
    out[..., jj % 16, jj // 16] = vals.astype(np.int16)
    return np.tile(out, lead and (1, 8, 1) or (8, 1))


# ----------------------------------------------------------------- host prep
def _prep(inputs):
    x = np.asarray(inputs["x"], np.float32)
    ei = np.asarray(inputs["edge_index"])
    ea = np.asarray(inputs["edge_attr"], np.float32)
    batch = np.asarray(inputs["batch"])
    W1 = np.asarray(inputs["W1"], np.float32)
    We1 = np.asarray(inputs["We1"], np.float32)
    a_src1 = np.asarray(inputs["a_src1"], np.float32)
    a_dst1 = np.asarray(inputs["a_dst1"], np.float32)
    a_edge1 = np.asarray(inputs["a_edge1"], np.float32)
    b1 = np.asarray(inputs["b1"], np.float32)
    ln1_w = np.asarray(inputs["ln1_w"], np.float32)
    ln1_b = np.asarray(inputs["ln1_b"], np.float32)
    W2 = np.asarray(inputs["W2"], np.float32)
    We2 = np.asarray(inputs["We2"], np.float32)
    a_src2 = np.asarray(inputs["a_src2"], np.float32)
    a_dst2 = np.asarray(inputs["a_dst2"], np.float32)
    a_edge2 = np.asarray(inputs["a_edge2"], np.float32)
    b2 = np.asarray(inputs["b2"], np.float32)
    ln2_w = np.asarray(inputs["ln2_w"], np.float32)
    ln2_b = np.asarray(inputs["ln2_b"], np.float32)

    # edges + self loops, sorted by dst
    loop = np.arange(N, dtype=np.int64)
    src = np.concatenate([ei[0].astype(np.int64), loop])
    dst = np.concatenate([ei[1].astype(np.int64), loop])
    order = np.argsort(dst, kind="stable")
    src, dst = src[order], dst[order]
    ea_mean = ea.mean(0)
    ea_sorted = np.empty((len(src), ED), np.float32)
    is_loop = order >= E
    ea_sorted[~is_loop] = ea[order[~is_loop]]
    ea_sorted[is_loop] = ea_mean

    src_core = src // SH
    src_loc = src % SH
    src_row = src_core * PADN + src_loc          # row in the L1 full table
    is_lo = src_row < HALF
    # layer-2 pair-packed rows: local row j holds nodes j and j+HB2
    src_row2 = src_core * HB2 + src_loc % HB2
    src_par2 = src_loc // HB2

    # per (core, window) lo/hi counts -> global fixed block counts
    core_of = dst // SH
    win_of = (dst - core_of * SH) // P
    gwin = core_of * WPC + win_of
    nlo = np.bincount(gwin[is_lo], minlength=NCORES * WPC)
    nhi = np.bincount(gwin[~is_lo], minlength=NCORES * WPC)
    SLB = int(np.ceil(nlo.max() / P))
    SHB = int(np.ceil(nhi.max() / P))
    nbk = SLB + SHB
    slots = nbk * P

    counts = np.bincount(gwin, minlength=NCORES * WPC)
    starts = np.zeros(NCORES * WPC + 1, np.int64)
    np.cumsum(counts, out=starts[1:])

    # folded attention weight vectors
    u1s = (W1.reshape(FIN, H, C1) * a_src1[None]).sum(-1)
    u1d = (W1.reshape(FIN, H, C1) * a_dst1[None]).sum(-1)
    v1 = (We1.reshape(ED, H, C1) * a_edge1[None]).sum(-1)
    u2s = (W2.reshape(H * C1, 1, C2) * a_src2[None]).sum(-1)
    u2d = (W2.reshape(H * C1, 1, C2) * a_dst2[None]).sum(-1)
    v2 = (We2.reshape(ED, 1, C2) * a_edge2[None]).sum(-1)

    w1cat = np.concatenate([W1, u1s, u1d], 1)                 # [128, 264]
    w2cat = np.concatenate([W2, u2s, u2d], 1)                 # [256, 66]
    v1p = np.concatenate([v1, np.full((1, H), NEG, np.float32)], 0)
    v2p = np.concatenate([v2, np.full((1, 1), NEG, np.float32)], 0)

    rep = lambda v: np.broadcast_to(v[None, :], (P, v.shape[0])).copy()
    iota_row = np.broadcast_to(
        np.arange(P, dtype=np.float32)[None, :], (P, P)).astype(BF).copy()
    iotaP = np.arange(P, dtype=np.float32)[:, None].astype(BF).copy()
    iota64 = np.broadcast_to(
        np.arange(G, dtype=np.float32)[None, :], (P, G)).copy()

    shared = dict(
        w1cat=np.ascontiguousarray(w1cat, np.float32),
        w2a=np.ascontiguousarray(w2cat[:P]).astype(BF),
        w2b=np.ascontiguousarray(w2cat[P:]).astype(BF),
        v1p=v1p.astype(BF), v2p=v2p.astype(BF),
        b1r=rep(b1), ln1wr=rep(ln1_w), ln1br=rep(ln1_b),
        b2r=rep(b2), ln2wr=rep(ln2_w), ln2br=rep(ln2_b),
        iota_row=iota_row, iotaP=iotaP, iota64=iota64,
    )

    in_maps = []
    for c in range(NCORES):
        lo = c * SH
        xT = np.zeros((FIN, PADN), np.float32)
        xT[:, :SH] = x[lo:lo + SH].T
        glo = np.zeros((WPC, SLB * P), np.int64)
        ghi = np.zeros((WPC, SHB * P), np.int64)
        g2 = np.zeros((WPC, slots), np.int64)
        par2 = np.zeros((WPC, P, nbk), np.uint8)
        dcol = np.zeros((WPC, P, nbk), np.float32)
        dcolT = np.zeros((WPC, slots), np.float32)
        eaT = np.zeros((WPC, ED + 1, slots), np.float32)
        eaT[:, ED, :] = 1.0  # pad flag on by default
        bcol = np.full((WPC, P), 999.0, np.float32)
        bcol.reshape(-1)[:SH] = batch[lo:lo + SH].astype(np.float32)

        for w in range(WPC):
            g = c * WPC + w
            s, e = starts[g], starts[g + 1]
            if e == s:
                continue
            sr = src_row[s:e]
            sr2 = src_row2[s:e]
            sp2 = src_par2[s:e]
            ed_ = dst[s:e]
            eaw = ea_sorted[s:e]
            ml = sr < HALF
            for half, msel, base, tab in ((0, ml, 0, glo),
                                          (1, ~ml, SLB, ghi)):
                idxs = np.nonzero(msel)[0]
                n_h = len(idxs)
                if n_h == 0:
                    continue
                jj = np.arange(n_h)
                pp, kk = jj % P, base + jj // P
                slot = kk * P + pp
                tab[w, jj] = sr[idxs] - (HALF if half else 0)
                g2[w, slot] = sr2[idxs]
                par2[w, pp, kk] = sp2[idxs].astype(np.uint8)
                dc = (ed_[idxs] - lo - w * P).astype(np.float32)
                dcol[w, pp, kk] = dc
                dcolT[w, slot] = dc
                eaT[w, :ED, slot] = eaw[idxs]  # adv-index: slot axis first
                eaT[w, ED, slot] = 0.0

        m = dict(shared)
        m.update(
            xT=xT,
            glo=_wrap16(glo), ghi=_wrap16(ghi), g2=_wrap16(g2),
            dcol=dcol.astype(BF), par2=par2,
            dcolT=np.broadcast_to(dcolT[:, None, :].astype(BF),
                                  (WPC, P, slots)).copy(),
            eaT=eaT.astype(BF), bcol=bcol,
        )
        in_maps.append(m)
    return in_maps, (SLB, SHB)


# ------------------------------------------------------------- device program
def _build(blocks):
    SLB, SHB = blocks
    nbk = SLB + SHB
    slots = nbk * P
    nc = bacc.Bacc("TRN2", target_bir_lowering=False, debug=False,
                   num_devices=NCORES)
    rg = [list(range(NCORES))]

    t_in = {}
    for name, shape, dt in [
        ("xT", [FIN, PADN], F32),
        ("w1cat", [FIN, 264], F32),
        ("w2a", [P, C2 + 2], BF16), ("w2b", [P, C2 + 2], BF16),
        ("v1p", [ED + 1, H], BF16), ("v2p", [ED + 1, 1], BF16),
        ("b1r", [P, H * C1], F32), ("ln1wr", [P, H * C1], F32),
        ("ln1br", [P, H * C1], F32),
        ("b2r", [P, C2], F32), ("ln2wr", [P, C2], F32),
        ("ln2br", [P, C2], F32),
        ("iota_row", [P, P], BF16), ("iotaP", [P, 1], BF16),
        ("iota64", [P, G], F32),
        ("glo", [WPC, P, SLB * P // 16], I16),
        ("ghi", [WPC, P, SHB * P // 16], I16),
        ("g2", [WPC, P, slots // 16], I16),
        ("dcol", [WPC, P, nbk], BF16),
        ("dcolT", [WPC, P, slots], BF16),
        ("par2", [WPC, P, nbk], U8),
        ("eaT", [WPC, ED + 1, slots], BF16),
        ("bcol", [WPC, P], F32),
    ]:
        t_in[name] = nc.dram_tensor(name, shape, dt, kind="ExternalInput")
    out_partial = nc.dram_tensor("partial", [G, G + 1], F32,
                                 kind="ExternalOutput")

    with tile.TileContext(nc) as tc, ExitStack() as ctx:
        const = ctx.enter_context(tc.tile_pool(name="const", bufs=1))
        work = ctx.enter_context(tc.tile_pool(name="work", bufs=3))
        big = ctx.enter_context(tc.tile_pool(name="big", bufs=1))
        psum = ctx.enter_context(tc.tile_pool(name="psum", bufs=2,
                                              space="PSUM"))
        dram = ctx.enter_context(tc.tile_pool(name="dram", bufs=1,
                                              space="DRAM"))

        # const APs used as activation bias operands
        zero_t = const.tile([P, 1], F32)
        nc.vector.memset(zero_t[:], 0.0)
        nc.const_aps.aps[(F32, 0.0)] = zero_t[:]
        eps_t = const.tile([P, 1], F32)
        nc.vector.memset(eps_t[:], EPS)
        nc.const_aps.aps[(F32, EPS)] = eps_t[:]

        def cload(name, dt=F32):
            src_t = t_in[name]
            t = const.tile(list(src_t.shape), dt, name=f"c_{name}")
            nc.sync.dma_start(t[:], src_t.ap())
            return t

        w1cat_sb = cload("w1cat")
        w2a_sb = cload("w2a", BF16)
        w2b_sb = cload("w2b", BF16)
        v1p_sb = cload("v1p", BF16)
        v2p_sb = cload("v2p", BF16)
        b1_sb = cload("b1r"); ln1w_sb = cload("ln1wr"); ln1b_sb = cload("ln1br")
        b2_sb = cload("b2r"); ln2w_sb = cload("ln2wr"); ln2b_sb = cload("ln2br")
        iota_row_sb = cload("iota_row", BF16)
        iotaP_sb = cload("iotaP", BF16)
        iota64_sb = cload("iota64")
        from concourse.masks import make_identity
        ident_sb = const.tile([P, P], F32)
        make_identity(nc, ident_sb[:])

        # small per-window tables, loaded once
        dcol_all = const.tile([P, WPC, nbk], BF16)
        nc.sync.dma_start(dcol_all[:],
                          t_in["dcol"].ap().rearrange("w p k -> p w k"))
        par2_all = const.tile([P, WPC, nbk], U8)
        nc.sync.dma_start(par2_all[:],
                          t_in["par2"].ap().rearrange("w p k -> p w k"))
        bcol_all = const.tile([P, WPC], F32)
        nc.sync.dma_start(bcol_all[:],
                          t_in["bcol"].ap().rearrange("w p -> p w"))
        ilo_all = big.tile([P, WPC, SLB * P // 16], I16)
        nc.sync.dma_start(ilo_all[:],
                          t_in["glo"].ap().rearrange("w p c -> p w c"))
        ihi_all = big.tile([P, WPC, SHB * P // 16], I16)
        nc.sync.dma_start(ihi_all[:],
                          t_in["ghi"].ap().rearrange("w p c -> p w c"))
        ig2_all = big.tile([P, WPC, slots // 16], I16)
        nc.sync.dma_start(ig2_all[:],
                          t_in["g2"].ap().rearrange("w p c -> p w c"))

        # DRAM scratch
        bounce1 = dram.tile([PADN, ROW1], U8)
        table1 = dram.tile([NCORES * PADN, ROW1], U8, addr_space="Shared")
        bounce2 = dram.tile([HB2, ROW2], U8)
        table2 = dram.tile([NCORES * HB2, ROW2], U8, addr_space="Shared")

        stage1 = big.tile([P, WPC, 264], U8)
        ald1_all = big.tile([P, WPC, H], BF16)
        z01_all = big.tile([P, WPC, nbk * H], BF16)
        accum1 = big.tile([P, WPC, 260], F32)
        stage2 = big.tile([P, WPC, P], U8)
        nc.vector.memset(stage2[:], 0)
        ald2_all = big.tile([P, WPC, 1], BF16)
        z02_all = big.tile([P, WPC, nbk], BF16)
        accum2 = big.tile([P, WPC, C2 + 1], F32)
        h3_sb = big.tile([P, WPC, G + 1], F32)

        # ---------------- phase 0: h_aug = x @ [W1|u1s|u1d] -> stage1
        XG = 4
        for w in range(WPC):
            if w % XG == 0:
                gn = min(XG, WPC - w)
                xg = work.tile([FIN, XG * P], F32, tag="xg", bufs=2)
                nc.sync.dma_start(
                    xg[:, 0:gn * P],
                    t_in["xT"].ap()[:, w * P:(w + gn) * P])
            ps0 = psum.tile([P, 264], F32, tag="mm", bufs=3)
            nc.tensor.matmul(ps0[:], lhsT=xg[:, (w % XG) * P:(w % XG + 1) * P],
                             rhs=w1cat_sb[:], start=True, stop=True)
            nc.scalar.activation(stage1[:, w, 0:256].bitcast(F8), ps0[:, 0:256],
                                 AF.Copy)
            nc.scalar.activation(stage1[:, w, 256:264].bitcast(BF16),
                                 ps0[:, 256:260], AF.Copy)
            nc.scalar.activation(ald1_all[:, w, :], ps0[:, 260:264], AF.Copy)
        nc.sync.dma_start(
            bounce1[:].rearrange("(w p) c -> p w c", p=P)[:, :, 0:264],
            stage1[:])
        nc.gpsimd.collective_compute(
            "AllGather", OP.bypass, replica_groups=rg,
            ins=[bounce1.opt()], outs=[table1.opt()])

        # ---------------- edge prep: al_edge + al_dst logits per window
        def edge_prep(nh, vp_sb, ald_all, z0_all):
            for g0 in range(0, WPC, EGRP):
                gn = min(EGRP, WPC - g0)
                eat = work.tile([ED + 1, gn, slots], BF16, tag="eat",
                                bufs=2)
                nc.sync.dma_start(
                    eat[:],
                    t_in["eaT"].ap()[g0:g0 + gn].rearrange(
                        "w r s -> r w s"))
                dcT = work.tile([P, gn, slots], BF16, tag="dcT", bufs=2)
                nc.sync.dma_start(
                    dcT[:],
                    t_in["dcolT"].ap()[g0:g0 + gn].rearrange(
                        "w p s -> p w s"))
                for wi in range(gn):
                    w = g0 + wi
                    ohT = work.tile([P, slots], BF16, tag="ohT", bufs=2)
                    nc.vector.scalar_tensor_tensor(
                        out=ohT[:], in0=dcT[:, wi], scalar=0.0,
                        in1=iotaP_sb[:].to_broadcast([P, slots]),
                        op0=OP.add, op1=OP.is_equal)
                    zz = psum.tile([P, nbk * nh], F32, tag="zps", bufs=2)
                    for k in range(nbk):
                        nc.tensor.matmul(
                            zz[:, k * nh:(k + 1) * nh],
                            lhsT=eat[:, wi, k * P:(k + 1) * P],
                            rhs=vp_sb[:], start=True, stop=False)
                        nc.tensor.matmul(
                            zz[:, k * nh:(k + 1) * nh],
                            lhsT=ohT[:, k * P:(k + 1) * P],
                            rhs=ald_all[:, w, 0:nh],
                            start=False, stop=True)
                    nc.scalar.activation(z0_all[:, w, :], zz[:], AF.Copy)

        edge_prep(H, v1p_sb, ald1_all, z01_all)

        # ---------------- L1 consume: gather + softmax + scatter
        def build_oh(w):
            oh = work.tile([P, nbk, P], BF16, tag="oh", bufs=3)
            nc.vector.scalar_tensor_tensor(
                out=oh[:],
                in0=iota_row_sb[:].rearrange("p (o j) -> p o j", o=1)
                    .to_broadcast([P, nbk, P]),
                scalar=0.0,
                in1=dcol_all[:, w].rearrange("p (k o) -> p k o", o=1)
                    .to_broadcast([P, nbk, P]),
                op0=OP.add, op1=OP.is_equal)
            return oh

        for w in range(WPC):
            gb = work.tile([P, nbk, ROW1], U8, tag="gb", bufs=2)
            nc.gpsimd.dma_gather(
                out_ap=gb[:, 0:SLB, :], in_ap=table1[0:HALF, :],
                idxs_ap=ilo_all[:, w], num_idxs=SLB * P,
                num_idxs_reg=SLB * P, elem_size=ROW1, single_packet=False)
            nc.gpsimd.dma_gather(
                out_ap=gb[:, SLB:nbk, :], in_ap=table1[HALF:2 * HALF, :],
                idxs_ap=ihi_all[:, w], num_idxs=SHB * P,
                num_idxs_reg=SHB * P, elem_size=ROW1, single_packet=False)
            oh = build_oh(w)
            z = work.tile([P, nbk, H], F32, tag="z")
            nc.vector.scalar_tensor_tensor(
                out=z[:], in0=gb[:, :, 256:264].bitcast(BF16), scalar=0.0,
                in1=z01_all[:, w].rearrange("p (k h) -> p k h", h=H),
                op0=OP.add, op1=OP.add)
            nc.vector.scalar_tensor_tensor(
                out=z[:], in0=z[:], scalar=0.2, in1=z[:],
                op0=OP.mult, op1=OP.max)
            pexp = work.tile([P, nbk, H], BF16, tag="pexp")
            nc.scalar.activation(pexp[:], z[:], AF.Exp)
            msg = work.tile([P, nbk, 260], BF16, tag="msg", bufs=2)
            nc.vector.scalar_tensor_tensor(
                out=msg[:, :, 0:256].rearrange("p k (h c) -> p k h c", h=H),
                in0=gb[:, :, 0:256].bitcast(F8)
                    .rearrange("p k (h c) -> p k h c", h=H),
                scalar=1.0,
                in1=pexp[:].rearrange("p k (h o) -> p k h o", o=1)
                    .to_broadcast([P, nbk, H, C1]),
                op0=OP.mult, op1=OP.mult)
            nc.vector.tensor_copy(msg[:, :, 256:260], pexp[:])
            sc = psum.tile([P, 260], F32, tag="sc", bufs=2)
            for k in range(nbk):
                nc.tensor.matmul(sc[:], lhsT=oh[:, k, :], rhs=msg[:, k, :],
                                 start=(k == 0), stop=(k == nbk - 1))
            nc.scalar.activation(accum1[:, w, :], sc[:], AF.Copy)

        # ---------------- normalize + relu + LN
        def norm_ln(acc, nh, feat, y_out, b_sb, lnw_sb, lnb_sb):
            rec = work.tile([P, nh], F32, tag="rec")
            nc.vector.tensor_scalar_add(rec[:], acc[:, feat:feat + nh], 1e-16)
            nc.vector.reciprocal(rec[:], rec[:])
            nc.vector.scalar_tensor_tensor(
                out=y_out.rearrange("p (h c) -> p h c", h=nh),
                in0=acc[:, 0:feat].rearrange("p (h c) -> p h c", h=nh),
                scalar=1.0,
                in1=rec[:].rearrange("p (h o) -> p h o", o=1)
                    .to_broadcast([P, nh, feat // nh]),
                op0=OP.mult, op1=OP.mult)
            nc.vector.scalar_tensor_tensor(
                out=y_out, in0=y_out, scalar=0.0, in1=b_sb[:, 0:feat],
                op0=OP.add, op1=OP.add)
            nc.scalar.activation(y_out, y_out, AF.Relu)
            mu = work.tile([P, 1], F32, tag="mu")
            nc.vector.tensor_reduce(mu[:], y_out, axis=mybir.AxisListType.X,
                                    op=OP.add)
            mus = work.tile([P, 1], F32, tag="mus")
            nc.scalar.activation(mus[:], mu[:], AF.Copy, scale=1.0 / feat)
            nc.vector.tensor_scalar_sub(y_out, y_out, mus[:, 0:1])
            sq = work.tile([P, feat], F32, tag="sq", bufs=2)
            var = work.tile([P, 1], F32, tag="var")
            nc.scalar.activation(sq[:], y_out, AF.Square, accum_out=var[:])
            isd = work.tile([P, 1], F32, tag="isd")
            nc.scalar.activation(isd[:], var[:], AF.Ln, bias=EPS,
                                 scale=1.0 / feat)
            nc.scalar.activation(isd[:], isd[:], AF.Exp, scale=-0.5)
            nc.vector.scalar_tensor_tensor(
                out=y_out, in0=y_out, scalar=isd[:, 0:1], in1=lnw_sb[:, 0:feat],
                op0=OP.mult, op1=OP.mult)
            nc.vector.scalar_tensor_tensor(
                out=y_out, in0=y_out, scalar=0.0, in1=lnb_sb[:, 0:feat],
                op0=OP.add, op1=OP.add)

        for w in range(WPC):
            norm_ln(accum1[:, w], H, H * C1, accum1[:, w, 0:H * C1], b1_sb,
                    ln1w_sb, ln1b_sb)

        # ---------------- layer-2 prep: [h2 | als2 | ald2] = h @ w2cat
        for w in range(WPC):
            ps2 = psum.tile([P, C2 + 2], F32, tag="mm", bufs=3)
            for fb in range(2):
                tp = psum.tile([P, P], F32, tag="mm", bufs=3)
                nc.tensor.transpose(tp[:], accum1[:, w, fb * P:(fb + 1) * P],
                                    ident_sb[:])
                tsb = work.tile([P, P], BF16, tag="tsb")
                nc.scalar.activation(tsb[:], tp[:], AF.Copy)
                nc.tensor.matmul(ps2[:], lhsT=tsb[:],
                                 rhs=(w2a_sb[:] if fb == 0 else w2b_sb[:]),
                                 start=(fb == 0), stop=(fb == 1))
            nc.scalar.activation(stage2[:, w, 0:C2].bitcast(F8), ps2[:, 0:C2],
                                 AF.Copy)
            nc.scalar.activation(stage2[:, w, C2:C2 + 2].bitcast(BF16),
                                 ps2[:, C2:C2 + 1], AF.Copy)
            nc.scalar.activation(ald2_all[:, w, :], ps2[:, C2 + 1:C2 + 2],
                                 AF.Copy)
        # pair-packed bounce2 writes: node i -> row i%HB2, col-half i//HB2
        WLO = HB2 // P  # 24 full lo windows, then window 24 straddles
        nc.sync.dma_start(
            bounce2[0:WLO * P, 0:P].rearrange("(w p) c -> p w c", p=P),
            stage2[:, 0:WLO, :])
        nc.sync.dma_start(bounce2[WLO * P:HB2, 0:P],
                          stage2[0:P // 2, WLO, :])
        nc.sync.dma_start(bounce2[0:P // 2, P:ROW2],
                          stage2[P // 2:P, WLO, :])
        nc.sync.dma_start(
            bounce2[P // 2:HB2, P:ROW2]
            .rearrange("(w p) c -> p w c", p=P),
            stage2[:, WLO + 1:WPC, :])
        nc.gpsimd.collective_compute(
            "AllGather", OP.bypass, replica_groups=rg,
            ins=[bounce2.opt()], outs=[table2.opt()])

        edge_prep(1, v2p_sb, ald2_all, z02_all)

        # ---------------- L2 consume
        for w in range(WPC):
            gb2 = work.tile([P, nbk, ROW2], U8, tag="gb2", bufs=2)
            nc.gpsimd.dma_gather(
                out_ap=gb2[:], in_ap=table2[0:NCORES * HB2, :],
                idxs_ap=ig2_all[:, w], num_idxs=slots,
                num_idxs_reg=slots, elem_size=ROW2, single_packet=False)
            gb2_bf = gb2[:].bitcast(BF16)  # [P, nbk, 128]
            # parity-select the pair half in place into the lo half
            nc.vector.copy_predicated(
                gb2_bf[:, :, 0:33],
                par2_all[:, w].rearrange("p (k o) -> p k o", o=1)
                    .to_broadcast([P, nbk, 33]),
                gb2_bf[:, :, 64:97])
            sel = gb2_bf
            oh = build_oh(w)
            z = work.tile([P, nbk, 1], F32, tag="z2")
            nc.vector.scalar_tensor_tensor(
                out=z[:], in0=sel[:, :, 32:33], scalar=0.0,
                in1=z02_all[:, w].rearrange("p (k o) -> p k o", o=1),
                op0=OP.add, op1=OP.add)
            nc.vector.scalar_tensor_tensor(
                out=z[:], in0=z[:], scalar=0.2, in1=z[:],
                op0=OP.mult, op1=OP.max)
            pexp = work.tile([P, nbk, 1], BF16, tag="pexp2")
            nc.scalar.activation(pexp[:], z[:], AF.Exp)
            msg = work.tile([P, nbk, C2 + 1], BF16, tag="msg2", bufs=2)
            nc.vector.scalar_tensor_tensor(
                out=msg[:, :, 0:C2],
                in0=sel[:, :, 0:32].bitcast(F8),
                scalar=1.0,
                in1=pexp[:].to_broadcast([P, nbk, C2]),
                op0=OP.mult, op1=OP.mult)
            nc.vector.tensor_copy(msg[:, :, C2:C2 + 1], pexp[:])
            sc = psum.tile([P, C2 + 1], F32, tag="sc", bufs=2)
            for k in range(nbk):
                nc.tensor.matmul(sc[:], lhsT=oh[:, k, :], rhs=msg[:, k, :],
                                 start=(k == 0), stop=(k == nbk - 1))
            nc.scalar.activation(accum2[:, w, :], sc[:], AF.Copy)

        nc.vector.memset(h3_sb[:], 1.0)
        for w in range(WPC):
            norm_ln(accum2[:, w], 1, C2, h3_sb[:, w, 0:C2], b2_sb, ln2w_sb,
                    ln2b_sb)

        # ---------------- graph mean-pool partials
        pl = psum.tile([G, G + 1], F32, tag="pl", bufs=1)
        for w in range(WPC):
            bh = work.tile([P, G], F32, tag="bh")
            nc.vector.tensor_scalar(
                out=bh[:], in0=iota64_sb[:], scalar1=bcol_all[:, w:w + 1],
                scalar2=None, op0=OP.is_equal)
            nc.tensor.matmul(pl[:], lhsT=bh[:], rhs=h3_sb[:, w, :],
                             start=(w == 0), stop=(w == WPC - 1))
        plo = work.tile([G, G + 1], F32)
        nc.vector.tensor_copy(plo[:], pl[:])
        nc.sync.dma_start(out_partial.ap(), plo[:])

    nc.compile()
    return nc


_CACHE = {}


def _get_program(blocks):
    if blocks not in _CACHE:
        _CACHE[blocks] = _build(blocks)
    return _CACHE[blocks]


def _run(inputs, trace=False):
    in_maps, blocks = _prep(inputs)
    nc = _get_program(blocks)
    res = run_bass_kernel_spmd(nc, in_maps, core_ids=list(range(NCORES)),
                               trace=trace)
    total = np.zeros((G, G + 1), np.float64)
    for c in range(NCORES):
        total += res.results[c]["partial"].astype(np.float64)
    out = total[:, :G] / np.maximum(total[:, G:G + 1], 1.0)
    return out.astype(np.float32), res


def kernel(**inputs):
    out, _ = _run(inputs, trace=False)
    return out


# revision 20
# speedup vs baseline: 1.5332x; 1.3039x over previous
"""GAT (2-layer, PyG GATConv-style) on 8 Trainium2 NeuronCores.

Strategy (dst-partitioned message passing, memory-optimized):
  - Nodes split into 8 shards of 6250 (padded to 6272 = 49*128 per core).
  - Edges (incl. self-loops) sorted by dst, routed to the dst-owning core,
    grouped into 49 windows of 128 dst nodes, each window padded to fixed
    128-edge blocks (lo/hi split by src half so gather indices fit int16).
  - Layer 1 table rows are 512B: 256 fp8e4m3 h values + 4 bf16 al_src
    scores; the table is AllGathered (25.7MB vs 64MB in f32) and each core
    dma_gathers source rows per edge.
  - Layer 2 table rows are 256B and pack TWO nodes ([64 fp8 h + 1 bf16
    al_src] each); the consumer selects the half by parity, halving the
    AllGather to 6.4MB.
  - al_dst is applied via a transposed one-hot matmul on the tensor engine
    (al_dst of a window's 128 dst nodes stays in SBUF) instead of a
    256B-per-edge gather.
  - Edge-attr loads, al_edge matmuls and one-hot builds are issued between
    the AllGather and the table gathers so they execute during the
    collective.
  - Per-edge: p = exp(leakyrelu(al_src+al_dst+al_edge)) (max-shift skipped;
    logits are O(1)); messages scaled by p on DVE (scalar_tensor_tensor,
    all-SBUF 2x mode) and scatter-added per dst window via bf16 one-hot
    matmuls; softmax denominator rides as extra matmul columns.
  - LayerNorm rsqrt via Ln+Exp so every activation lives in one act table.
  - Graph mean-pool via batch-one-hot matmul; host sums 8 partial [64,65]
    outputs and divides.

Host does only index bookkeeping and small-weight folding; all O(N*F) /
O(E*F) floating-point math runs on device.
"""

import sys

for _p in ("/opt/trn_rl_repo",):
    if _p not in sys.path:
        sys.path.insert(0, _p)

from contextlib import ExitStack

import ml_dtypes
import numpy as np

import concourse.bass as bass
import concourse.mybir as mybir
import concourse.tile as tile
from concourse import bacc
from concourse.bass_utils import run_bass_kernel_spmd

F32 = mybir.dt.float32
BF16 = mybir.dt.bfloat16
F8 = mybir.dt.float8e4
U8 = mybir.dt.uint8
I16 = mybir.dt.int16
AF = mybir.ActivationFunctionType
OP = mybir.AluOpType
BF = ml_dtypes.bfloat16

NCORES = 8
N, E, FIN, ED = 50000, 400000, 128, 6
H, C1, C2, G = 4, 64, 64, 64
EPS = 1e-5
P = 128
SH = N // NCORES            # 6250 nodes per shard
WPC = (SH + P - 1) // P     # 49 windows per core
PADN = WPC * P              # 6272 padded shard rows
HALF = 4 * PADN             # 25088 rows per half-table (int16-safe)
HB2 = PADN // 2             # 3136 pair rows per core (layer-2 table)
ROW1 = 512                  # L1 table row: 256 fp8 h + 4 bf16 al_src
ROW2 = 256                  # L2 table row bytes: 2x(64 fp8 h + 1 bf16 al)
NEG = -1.0e9
EGRP = 1                    # windows per eaT load


def _wrap16(vals):
    """[..., L] int -> [..., 128, L//16] int16, idx j at (j%16, j//16),
    replicated across the 8 gpsimd core windows."""
    lead = vals.shape[:-1]
    L = vals.shape[-1]
    out = np.zeros(lead + (16, L // 16), np.int16)
    jj = np.arange(L)
    out[..., jj % 16, jj // 16] = vals.astype(np.int16)
    return np.tile(out, lead and (1, 8, 1) or (8, 1))


# ----------------------------------------------------------------- host prep
def _prep(inputs):
    x = np.asarray(inputs["x"], np.float32)
    ei = np.asarray(inputs["edge_index"])
    ea = np.asarray(inputs["edge_attr"], np.float32)
    batch = np.asarray(inputs["batch"])
    W1 = np.asarray(inputs["W1"], np.float32)
    We1 = np.asarray(inputs["We1"], np.float32)
    a_src1 = np.asarray(inputs["a_src1"], np.float32)
    a_dst1 = np.asarray(inputs["a_dst1"], np.float32)
    a_edge1 = np.asarray(inputs["a_edge1"], np.float32)
    b1 = np.asarray(inputs["b1"], np.float32)
    ln1_w = np.asarray(inputs["ln1_w"], np.float32)
    ln1_b = np.asarray(inputs["ln1_b"], np.float32)
    W2 = np.asarray(inputs["W2"], np.float32)
    We2 = np.asarray(inputs["We2"], np.float32)
    a_src2 = np.asarray(inputs["a_src2"], np.float32)
    a_dst2 = np.asarray(inputs["a_dst2"], np.float32)
    a_edge2 = np.asarray(inputs["a_edge2"], np.float32)
    b2 = np.asarray(inputs["b2"], np.float32)
    ln2_w = np.asarray(inputs["ln2_w"], np.float32)
    ln2_b = np.asarray(inputs["ln2_b"], np.float32)

    # edges + self loops, sorted by dst
    loop = np.arange(N, dtype=np.int64)
    src = np.concatenate([ei[0].astype(np.int64), loop])
    dst = np.concatenate([ei[1].astype(np.int64), loop])
    order = np.argsort(dst, kind="stable")
    src, dst = src[order], dst[order]
    ea_mean = ea.mean(0)
    ea_sorted = np.empty((len(src), ED), np.float32)
    is_loop = order >= E
    ea_sorted[~is_loop] = ea[order[~is_loop]]
    ea_sorted[is_loop] = ea_mean

    src_core = src // SH
    src_loc = src % SH
    src_row = src_core * PADN + src_loc          # row in the L1 full table
    is_lo = src_row < HALF
    # layer-2 pair-packed rows: local row j holds nodes j and j+HB2
    src_row2 = src_core * HB2 + src_loc % HB2
    src_par2 = src_loc // HB2

    # per (core, window) lo/hi counts -> global fixed block counts
    core_of = dst // SH
    win_of = (dst - core_of * SH) // P
    gwin = core_of * WPC + win_of
    nlo = np.bincount(gwin[is_lo], minlength=NCORES * WPC)
    nhi = np.bincount(gwin[~is_lo], minlength=NCORES * WPC)
    SLB = int(np.ceil(nlo.max() / P))
    SHB = int(np.ceil(nhi.max() / P))
    nbk = SLB + SHB
    slots = nbk * P

    counts = np.bincount(gwin, minlength=NCORES * WPC)
    starts = np.zeros(NCORES * WPC + 1, np.int64)
    np.cumsum(counts, out=starts[1:])

    # folded attention weight vectors
    u1s = (W1.reshape(FIN, H, C1) * a_src1[None]).sum(-1)
    u1d = (W1.reshape(FIN, H, C1) * a_dst1[None]).sum(-1)
    v1 = (We1.reshape(ED, H, C1) * a_edge1[None]).sum(-1)
    u2s = (W2.reshape(H * C1, 1, C2) * a_src2[None]).sum(-1)
    u2d = (W2.reshape(H * C1, 1, C2) * a_dst2[None]).sum(-1)
    v2 = (We2.reshape(ED, 1, C2) * a_edge2[None]).sum(-1)

    w1cat = np.concatenate([W1, u1s, u1d], 1)                 # [128, 264]
    w2cat = np.concatenate([W2, u2s, u2d], 1)                 # [256, 66]
    v1p = np.concatenate([v1, np.full((1, H), NEG, np.float32)], 0)
    v2p = np.concatenate([v2, np.full((1, 1), NEG, np.float32)], 0)

    rep = lambda v: np.broadcast_to(v[None, :], (P, v.shape[0])).copy()
    iota_row = np.broadcast_to(
        np.arange(P, dtype=np.float32)[None, :], (P, P)).astype(BF).copy()
    iotaP2 = np.repeat(np.arange(P, dtype=np.float32)[:, None], 2,
                       1).astype(BF).copy()
    iota64 = np.broadcast_to(
        np.arange(G, dtype=np.float32)[None, :], (P, G)).copy()

    shared = dict(
        w1cat=np.ascontiguousarray(w1cat, np.float32),
        w2a=np.ascontiguousarray(w2cat[:P]).astype(BF),
        w2b=np.ascontiguousarray(w2cat[P:]).astype(BF),
        v1p=v1p.astype(BF), v2p=v2p.astype(BF),
        b1r=rep(b1), ln1wr=rep(ln1_w), ln1br=rep(ln1_b),
        b2r=rep(b2), ln2wr=rep(ln2_w), ln2br=rep(ln2_b),
        iota_row=iota_row, iotaP2=iotaP2, iota64=iota64,
    )
    triv = (not b1.any() and not ln1_b.any() and bool((ln1_w == 1).all()),
            not b2.any() and not ln2_b.any() and bool((ln2_w == 1).all()))

    in_maps = []
    for c in range(NCORES):
        lo = c * SH
        xT = np.zeros((FIN, PADN), np.float32)
        xT[:, :SH] = x[lo:lo + SH].T
        glo = np.zeros((WPC, SLB * P), np.int64)
        ghi = np.zeros((WPC, SHB * P), np.int64)
        g2 = np.zeros((WPC, slots), np.int64)
        par2 = np.zeros((WPC, P, nbk), np.uint8)
        dcol = np.zeros((WPC, P, nbk), np.float32)
        dcolT = np.zeros((WPC, slots), np.float32)
        eaT = np.zeros((WPC, ED + 1, slots), np.float32)
        eaT[:, ED, :] = 1.0  # pad flag on by default
        bcol = np.full((WPC, P), 999.0, np.float32)
        bcol.reshape(-1)[:SH] = batch[lo:lo + SH].astype(np.float32)

        for w in range(WPC):
            g = c * WPC + w
            s, e = starts[g], starts[g + 1]
            if e == s:
                continue
            sr = src_row[s:e]
            sr2 = src_row2[s:e]
            sp2 = src_par2[s:e]
            ed_ = dst[s:e]
            eaw = ea_sorted[s:e]
            ml = sr < HALF
            for half, msel, base, tab in ((0, ml, 0, glo),
                                          (1, ~ml, SLB, ghi)):
                idxs = np.nonzero(msel)[0]
                n_h = len(idxs)
                if n_h == 0:
                    continue
                jj = np.arange(n_h)
                pp, kk = jj % P, base + jj // P
                slot = kk * P + pp
                tab[w, jj] = sr[idxs] - (HALF if half else 0)
                g2[w, slot] = sr2[idxs]
                par2[w, pp, kk] = sp2[idxs].astype(np.uint8)
                dc = (ed_[idxs] - lo - w * P).astype(np.float32)
                dcol[w, pp, kk] = dc
                dcolT[w, slot] = dc
                eaT[w, :ED, slot] = eaw[idxs]  # adv-index: slot axis first
                eaT[w, ED, slot] = 0.0

        m = dict(shared)
        m.update(
            xT=xT,
            glo=_wrap16(glo), ghi=_wrap16(ghi), g2=_wrap16(g2),
            dcol2=np.repeat(dcol.astype(BF)[..., None], 2, -1), par2=par2,
            dcolT=np.broadcast_to(dcolT[:, None, :].astype(BF),
                                  (WPC, P, slots)).copy(),
            eaT=eaT.astype(BF), bcol=bcol,
        )
        in_maps.append(m)
    return in_maps, (SLB, SHB) + triv


# ------------------------------------------------------------- device program
def _build(blocks):
    SLB, SHB, triv1, triv2 = blocks
    nbk = SLB + SHB
    slots = nbk * P
    nc = bacc.Bacc("TRN2", target_bir_lowering=False, debug=False,
                   num_devices=NCORES)
    rg = [list(range(NCORES))]

    t_in = {}
    for name, shape, dt in [
        ("xT", [FIN, PADN], F32),
        ("w1cat", [FIN, 264], F32),
        ("w2a", [P, C2 + 2], BF16), ("w2b", [P, C2 + 2], BF16),
        ("v1p", [ED + 1, H], BF16), ("v2p", [ED + 1, 1], BF16),
        ("b1r", [P, H * C1], F32), ("ln1wr", [P, H * C1], F32),
        ("ln1br", [P, H * C1], F32),
        ("b2r", [P, C2], F32), ("ln2wr", [P, C2], F32),
        ("ln2br", [P, C2], F32),
        ("iota_row", [P, P], BF16), ("iotaP2", [P, 2], BF16),
        ("iota64", [P, G], F32),
        ("glo", [WPC, P, SLB * P // 16], I16),
        ("ghi", [WPC, P, SHB * P // 16], I16),
        ("g2", [WPC, P, slots // 16], I16),
        ("dcol2", [WPC, P, nbk, 2], BF16),
        ("dcolT", [WPC, P, slots], BF16),
        ("par2", [WPC, P, nbk], U8),
        ("eaT", [WPC, ED + 1, slots], BF16),
        ("bcol", [WPC, P], F32),
    ]:
        t_in[name] = nc.dram_tensor(name, shape, dt, kind="ExternalInput")
    out_partial = nc.dram_tensor("partial", [G, G + 1], F32,
                                 kind="ExternalOutput")

    with tile.TileContext(nc) as tc, ExitStack() as ctx:
        const = ctx.enter_context(tc.tile_pool(name="const", bufs=1))
        work = ctx.enter_context(tc.tile_pool(name="work", bufs=3))
        big = ctx.enter_context(tc.tile_pool(name="big", bufs=1))
        psum = ctx.enter_context(tc.tile_pool(name="psum", bufs=2,
                                              space="PSUM"))
        dram = ctx.enter_context(tc.tile_pool(name="dram", bufs=1,
                                              space="DRAM"))

        # const APs used as activation bias operands
        zero_t = const.tile([P, 1], F32)
        nc.vector.memset(zero_t[:], 0.0)
        nc.const_aps.aps[(F32, 0.0)] = zero_t[:]
        eps_t = const.tile([P, 1], F32)
        nc.vector.memset(eps_t[:], EPS)
        nc.const_aps.aps[(F32, EPS)] = eps_t[:]

        def cload(name, dt=F32):
            src_t = t_in[name]
            t = const.tile(list(src_t.shape), dt, name=f"c_{name}")
            nc.sync.dma_start(t[:], src_t.ap())
            return t

        w1cat_sb = cload("w1cat")
        w2a_sb = cload("w2a", BF16)
        w2b_sb = cload("w2b", BF16)
        v1p_sb = cload("v1p", BF16)
        v2p_sb = cload("v2p", BF16)
        b1_sb = cload("b1r"); ln1w_sb = cload("ln1wr"); ln1b_sb = cload("ln1br")
        b2_sb = cload("b2r"); ln2w_sb = cload("ln2wr"); ln2b_sb = cload("ln2br")
        iota_row_sb = cload("iota_row", BF16)
        iotaP2_sb = cload("iotaP2", BF16)
        iota64_sb = cload("iota64")
        from concourse.masks import make_identity
        ident_sb = const.tile([P, P], F32)
        make_identity(nc, ident_sb[:])

        # small per-window tables, loaded once
        dcol_all = const.tile([P, WPC, nbk, 2], BF16)
        nc.sync.dma_start(dcol_all[:],
                          t_in["dcol2"].ap().rearrange("w p k d -> p w k d"))
        par2_all = const.tile([P, WPC, nbk], U8)
        nc.sync.dma_start(par2_all[:],
                          t_in["par2"].ap().rearrange("w p k -> p w k"))
        bcol_all = const.tile([P, WPC], F32)
        nc.sync.dma_start(bcol_all[:],
                          t_in["bcol"].ap().rearrange("w p -> p w"))
        ilo_all = big.tile([P, WPC, SLB * P // 16], I16)
        nc.sync.dma_start(ilo_all[:],
                          t_in["glo"].ap().rearrange("w p c -> p w c"))
        ihi_all = big.tile([P, WPC, SHB * P // 16], I16)
        nc.sync.dma_start(ihi_all[:],
                          t_in["ghi"].ap().rearrange("w p c -> p w c"))
        ig2_all = big.tile([P, WPC, slots // 16], I16)
        nc.sync.dma_start(ig2_all[:],
                          t_in["g2"].ap().rearrange("w p c -> p w c"))

        # DRAM scratch
        bounce1 = dram.tile([PADN, ROW1], U8)
        table1 = dram.tile([NCORES * PADN, ROW1], U8, addr_space="Shared")
        bounce2 = dram.tile([HB2, ROW2], U8)
        table2 = dram.tile([NCORES * HB2, ROW2], U8, addr_space="Shared")

        stage1 = big.tile([P, 25, 264], U8)
        ald1_all = big.tile([P, WPC, H], BF16)
        z01_all = big.tile([P, WPC, nbk * H], BF16)
        z02e_all = big.tile([P, WPC, nbk], BF16)
        var1_all = big.tile([P, WPC], F32)
        isd1_all = big.tile([P, WPC], F32)
        var2_all = big.tile([P, WPC], F32)
        isd2_all = big.tile([P, WPC], F32)
        accum1 = big.tile([P, WPC, 260], F32)
        stage2 = big.tile([P, WPC, P], U8)
        nc.vector.memset(stage2[:], 0)
        ald2_all = big.tile([P, WPC, 1], BF16)
        z02_all = big.tile([P, WPC, nbk], BF16)
        accum2 = big.tile([P, WPC, C2 + 1], F32)
        h3_sb = big.tile([P, WPC, G + 1], F32)

        # ---------------- phase 0: h_aug = x @ [W1|u1s|u1d] -> stage1
        XG = 4
        for half in range(2):
            w0, w1 = (0, 25) if half == 0 else (25, WPC)
            for w in range(w0, w1):
                if (w - w0) % XG == 0:
                    gn = min(XG, w1 - w)
                    xg = work.tile([FIN, XG * P], F32, tag="xg", bufs=2)
                    nc.sync.dma_start(
                        xg[:, 0:gn * P],
                        t_in["xT"].ap()[:, w * P:(w + gn) * P])
                ps0 = psum.tile([P, 264], F32, tag="mm", bufs=3)
                nc.tensor.matmul(
                    ps0[:], lhsT=xg[:, ((w - w0) % XG) * P:
                                    ((w - w0) % XG + 1) * P],
                    rhs=w1cat_sb[:], start=True, stop=True)
                nc.scalar.activation(stage1[:, w - w0, 0:256].bitcast(F8),
                                     ps0[:, 0:256], AF.Copy)
                nc.scalar.activation(stage1[:, w - w0, 256:264].bitcast(BF16),
                                     ps0[:, 256:260], AF.Copy)
                nc.scalar.activation(ald1_all[:, w, :], ps0[:, 260:264],
                                     AF.Copy)
            nc.sync.dma_start(
                bounce1[w0 * P:w1 * P, 0:264]
                .rearrange("(w p) c -> p w c", p=P),
                stage1[:, 0:w1 - w0])
        nc.gpsimd.collective_compute(
            "AllGather", OP.bypass, replica_groups=rg,
            ins=[bounce1.opt()], outs=[table1.opt()])

        # ---------------- edge prep: al_edge + al_dst logits per window
        def build_ohT(dcT, wi):
            ohT = work.tile([P, slots], BF16, tag="ohT", bufs=2)
            nc.vector.tensor_tensor(
                out=ohT[:].rearrange("p (s d) -> p s d", d=2),
                in0=dcT[:, wi].rearrange("p (s d) -> p s d", d=2),
                in1=iotaP2_sb[:].rearrange("p (o d) -> p o d", o=1)
                    .to_broadcast([P, slots // 2, 2]),
                op=OP.is_equal)
            return ohT

        # L1 edge prep (runs during AG1); also precomputes the al_edge part
        # of layer 2 so eaT is loaded only once.
        for w in range(WPC):
            eat = work.tile([ED + 1, slots], BF16, tag="eat", bufs=2)
            nc.sync.dma_start(eat[:], t_in["eaT"].ap()[w])
            dcT = work.tile([P, 1, slots], BF16, tag="dcT", bufs=2)
            nc.sync.dma_start(dcT[:], t_in["dcolT"].ap()[w:w + 1]
                              .rearrange("w p s -> p w s"))
            ohT = build_ohT(dcT, 0)
            zz = psum.tile([P, nbk * H], F32, tag="zps", bufs=2)
            for k in range(nbk):
                nc.tensor.matmul(
                    zz[:, k * H:(k + 1) * H],
                    lhsT=eat[:, k * P:(k + 1) * P],
                    rhs=v1p_sb[:], start=True, stop=False)
                nc.tensor.matmul(
                    zz[:, k * H:(k + 1) * H],
                    lhsT=ohT[:, k * P:(k + 1) * P],
                    rhs=ald1_all[:, w, :],
                    start=False, stop=True)
            nc.scalar.activation(z01_all[:, w, :], zz[:], AF.Copy)
            zz2 = psum.tile([P, nbk], F32, tag="zps", bufs=2)
            for k in range(nbk):
                nc.tensor.matmul(
                    zz2[:, k:k + 1],
                    lhsT=eat[:, k * P:(k + 1) * P],
                    rhs=v2p_sb[:], start=True, stop=True)
            nc.scalar.activation(z02e_all[:, w, :], zz2[:], AF.Copy)

        # ---------------- L1 consume: gather + softmax + scatter
        def build_oh(w):
            oh = work.tile([P, nbk, P], BF16, tag="oh", bufs=3)
            nc.vector.tensor_tensor(
                out=oh[:].rearrange("p k (a d) -> p k a d", d=2),
                in0=iota_row_sb[:].rearrange("p (o a d) -> p o a d", o=1, d=2)
                    .to_broadcast([P, nbk, P // 2, 2]),
                in1=dcol_all[:, w].rearrange("p k (o d) -> p k o d", o=1)
                    .to_broadcast([P, nbk, P // 2, 2]),
                op=OP.is_equal)
            return oh

        for w in range(WPC):
            gb = work.tile([P, nbk, ROW1], U8, tag="gb", bufs=2)
            nc.gpsimd.dma_gather(
                out_ap=gb[:, 0:SLB, :], in_ap=table1[0:HALF, :],
                idxs_ap=ilo_all[:, w], num_idxs=SLB * P,
                num_idxs_reg=SLB * P, elem_size=ROW1, single_packet=False)
            nc.gpsimd.dma_gather(
                out_ap=gb[:, SLB:nbk, :], in_ap=table1[HALF:2 * HALF, :],
                idxs_ap=ihi_all[:, w], num_idxs=SHB * P,
                num_idxs_reg=SHB * P, elem_size=ROW1, single_packet=False)
            oh = build_oh(w)
            z = work.tile([P, nbk, H], BF16, tag="z")
            nc.vector.tensor_tensor(
                out=z[:], in0=gb[:, :, 256:264].bitcast(BF16),
                in1=z01_all[:, w].rearrange("p (k h) -> p k h", h=H),
                op=OP.add)
            nc.vector.scalar_tensor_tensor(
                out=z[:], in0=z[:], scalar=0.2, in1=z[:],
                op0=OP.mult, op1=OP.max)
            pexp = work.tile([P, nbk, H], BF16, tag="pexp")
            nc.scalar.activation(pexp[:], z[:], AF.Exp)
            msg = work.tile([P, nbk, 260], BF16, tag="msg", bufs=2)
            nc.vector.tensor_tensor(
                out=msg[:, :, 0:256].rearrange("p k (h c) -> p k h c", h=H),
                in0=gb[:, :, 0:256].bitcast(F8)
                    .rearrange("p k (h c) -> p k h c", h=H),
                in1=pexp[:].rearrange("p k (h o) -> p k h o", o=1)
                    .to_broadcast([P, nbk, H, C1]),
                op=OP.mult)
            nc.vector.tensor_copy(msg[:, :, 256:260], pexp[:])
            sc = psum.tile([P, 260], F32, tag="sc", bufs=2)
            for k in range(nbk):
                nc.tensor.matmul(sc[:], lhsT=oh[:, k, :], rhs=msg[:, k, :],
                                 start=(k == 0), stop=(k == nbk - 1))
            nc.scalar.activation(accum1[:, w, :], sc[:], AF.Copy)

        # ---------------- normalize + relu + LN (batched Ln/Exp)
        def norm_a(acc, nh, feat, y_out, b_sb, var_all, w, triv):
            rec = work.tile([P, nh], F32, tag="rec")
            nc.vector.tensor_scalar_add(rec[:], acc[:, feat:feat + nh], 1e-16)
            nc.vector.reciprocal(rec[:], rec[:])
            nc.vector.scalar_tensor_tensor(
                out=y_out.rearrange("p (h c) -> p h c", h=nh),
                in0=acc[:, 0:feat].rearrange("p (h c) -> p h c", h=nh),
                scalar=1.0,
                in1=rec[:].rearrange("p (h o) -> p h o", o=1)
                    .to_broadcast([P, nh, feat // nh]),
                op0=OP.mult, op1=OP.mult)
            if not triv:
                nc.vector.scalar_tensor_tensor(
                    out=y_out, in0=y_out, scalar=0.0, in1=b_sb[:, 0:feat],
                    op0=OP.add, op1=OP.add)
            nc.scalar.activation(y_out, y_out, AF.Relu)
            mu = work.tile([P, 1], F32, tag="mu")
            nc.vector.tensor_reduce(mu[:], y_out, axis=mybir.AxisListType.X,
                                    op=OP.add)
            mus = work.tile([P, 1], F32, tag="mus")
            nc.scalar.activation(mus[:], mu[:], AF.Copy, scale=1.0 / feat)
            nc.vector.tensor_scalar_sub(y_out, y_out, mus[:, 0:1])
            sq = work.tile([P, feat], F32, tag="sq", bufs=2)
            nc.scalar.activation(sq[:], y_out, AF.Square,
                                 accum_out=var_all[:, w:w + 1])

        def norm_b(y_out, isd_all, w, lnw_sb, lnb_sb, feat, triv):
            nc.vector.tensor_scalar_mul(y_out, y_out, isd_all[:, w:w + 1])
            if not triv:
                nc.vector.scalar_tensor_tensor(
                    out=y_out, in0=y_out, scalar=1.0, in1=lnw_sb[:, 0:feat],
                    op0=OP.mult, op1=OP.mult)
                nc.vector.scalar_tensor_tensor(
                    out=y_out, in0=y_out, scalar=0.0, in1=lnb_sb[:, 0:feat],
                    op0=OP.add, op1=OP.add)

        for w in range(WPC):
            norm_a(accum1[:, w], H, H * C1, accum1[:, w, 0:H * C1], b1_sb,
                   var1_all, w, triv1)
        nc.scalar.activation(isd1_all[:], var1_all[:], AF.Ln, bias=EPS,
                             scale=1.0 / (H * C1))
        nc.scalar.activation(isd1_all[:], isd1_all[:], AF.Exp, scale=-0.5)
        for w in range(WPC):
            norm_b(accum1[:, w, 0:H * C1], isd1_all, w, ln1w_sb, ln1b_sb,
                   H * C1, triv1)

        # ---------------- layer-2 prep: [h2 | als2 | ald2] = h @ w2cat
        for w in range(WPC):
            ps2 = psum.tile([P, C2 + 2], F32, tag="mm", bufs=3)
            for fb in range(2):
                tp = psum.tile([P, P], F32, tag="mm", bufs=3)
                nc.tensor.transpose(tp[:], accum1[:, w, fb * P:(fb + 1) * P],
                                    ident_sb[:])
                tsb = work.tile([P, P], BF16, tag="tsb")
                nc.scalar.activation(tsb[:], tp[:], AF.Copy)
                nc.tensor.matmul(ps2[:], lhsT=tsb[:],
                                 rhs=(w2a_sb[:] if fb == 0 else w2b_sb[:]),
                                 start=(fb == 0), stop=(fb == 1))
            nc.scalar.activation(stage2[:, w, 0:C2].bitcast(F8), ps2[:, 0:C2],
                                 AF.Copy)
            nc.scalar.activation(stage2[:, w, C2:C2 + 2].bitcast(BF16),
                                 ps2[:, C2:C2 + 1], AF.Copy)
            nc.scalar.activation(ald2_all[:, w, :], ps2[:, C2 + 1:C2 + 2],
                                 AF.Copy)
        # pair-packed bounce2 writes: node i -> row i%HB2, col-half i//HB2
        WLO = HB2 // P  # 24 full lo windows, then window 24 straddles
        nc.sync.dma_start(
            bounce2[0:WLO * P, 0:P].rearrange("(w p) c -> p w c", p=P),
            stage2[:, 0:WLO, :])
        nc.sync.dma_start(bounce2[WLO * P:HB2, 0:P],
                          stage2[0:P // 2, WLO, :])
        nc.sync.dma_start(bounce2[0:P // 2, P:ROW2],
                          stage2[P // 2:P, WLO, :])
        nc.sync.dma_start(
            bounce2[P // 2:HB2, P:ROW2]
            .rearrange("(w p) c -> p w c", p=P),
            stage2[:, WLO + 1:WPC, :])
        nc.gpsimd.collective_compute(
            "AllGather", OP.bypass, replica_groups=rg,
            ins=[bounce2.opt()], outs=[table2.opt()])

        for w in range(WPC):
            dcT = work.tile([P, 1, slots], BF16, tag="dcT", bufs=2)
            nc.sync.dma_start(dcT[:], t_in["dcolT"].ap()[w:w + 1]
                              .rearrange("w p s -> p w s"))
            ohT = build_ohT(dcT, 0)
            zz = psum.tile([P, nbk], F32, tag="zps", bufs=2)
            for k in range(nbk):
                nc.tensor.matmul(
                    zz[:, k:k + 1], lhsT=ohT[:, k * P:(k + 1) * P],
                    rhs=ald2_all[:, w, :], start=True, stop=True)
            za = work.tile([P, nbk], BF16, tag="za")
            nc.scalar.activation(za[:], zz[:], AF.Copy)
            nc.vector.tensor_tensor(z02_all[:, w], za[:], z02e_all[:, w],
                                    OP.add)

        # ---------------- L2 consume
        for w in range(WPC):
            gb2 = work.tile([P, nbk, ROW2], U8, tag="gb2", bufs=2)
            nc.gpsimd.dma_gather(
                out_ap=gb2[:], in_ap=table2[0:NCORES * HB2, :],
                idxs_ap=ig2_all[:, w], num_idxs=slots,
                num_idxs_reg=slots, elem_size=ROW2, single_packet=False)
            gb2_bf = gb2[:].bitcast(BF16)  # [P, nbk, 128]
            # parity-select the pair half in place into the lo half
            nc.vector.copy_predicated(
                gb2_bf[:, :, 0:33],
                par2_all[:, w].rearrange("p (k o) -> p k o", o=1)
                    .to_broadcast([P, nbk, 33]),
                gb2_bf[:, :, 64:97])
            sel = gb2_bf
            oh = build_oh(w)
            z = work.tile([P, nbk, 1], BF16, tag="z2")
            nc.vector.tensor_tensor(
                out=z[:], in0=sel[:, :, 32:33],
                in1=z02_all[:, w].rearrange("p (k o) -> p k o", o=1),
                op=OP.add)
            nc.vector.scalar_tensor_tensor(
                out=z[:], in0=z[:], scalar=0.2, in1=z[:],
                op0=OP.mult, op1=OP.max)
            pexp = work.tile([P, nbk, 1], BF16, tag="pexp2")
            nc.scalar.activation(pexp[:], z[:], AF.Exp)
            msg = work.tile([P, nbk, C2 + 1], BF16, tag="msg2", bufs=2)
            nc.vector.scalar_tensor_tensor(
                out=msg[:, :, 0:C2],
                in0=sel[:, :, 0:32].bitcast(F8),
                scalar=1.0,
                in1=pexp[:].to_broadcast([P, nbk, C2]),
                op0=OP.mult, op1=OP.mult)
            nc.vector.tensor_copy(msg[:, :, C2:C2 + 1], pexp[:])
            sc = psum.tile([P, C2 + 1], F32, tag="sc", bufs=2)
            for k in range(nbk):
                nc.tensor.matmul(sc[:], lhsT=oh[:, k, :], rhs=msg[:, k, :],
                                 start=(k == 0), stop=(k == nbk - 1))
            nc.scalar.activation(accum2[:, w, :], sc[:], AF.Copy)

        nc.vector.memset(h3_sb[:], 1.0)
        for w in range(WPC):
            norm_a(accum2[:, w], 1, C2, h3_sb[:, w, 0:C2], b2_sb,
                   var2_all, w, triv2)
        nc.scalar.activation(isd2_all[:], var2_all[:], AF.Ln, bias=EPS,
                             scale=1.0 / C2)
        nc.scalar.activation(isd2_all[:], isd2_all[:], AF.Exp, scale=-0.5)
        for w in range(WPC):
            norm_b(h3_sb[:, w, 0:C2], isd2_all, w, ln2w_sb, ln2b_sb, C2,
                   triv2)

        # ---------------- graph mean-pool partials
        pl = psum.tile([G, G + 1], F32, tag="pl", bufs=1)
        for w in range(WPC):
            bh = work.tile([P, G], F32, tag="bh")
            nc.vector.tensor_scalar(
                out=bh[:], in0=iota64_sb[:], scalar1=bcol_all[:, w:w + 1],
                scalar2=None, op0=OP.is_equal)
            nc.tensor.matmul(pl[:], lhsT=bh[:], rhs=h3_sb[:, w, :],
                             start=(w == 0), stop=(w == WPC - 1))
        plo = work.tile([G, G + 1], F32)
        nc.vector.tensor_copy(plo[:], pl[:])
        nc.sync.dma_start(out_partial.ap(), plo[:])

    nc.compile()
    return nc


_CACHE = {}


def _get_program(blocks):
    if blocks not in _CACHE:
        _CACHE[blocks] = _build(blocks)
    return _CACHE[blocks]


def _run(inputs, trace=False):
    in_maps, blocks = _prep(inputs)
    nc = _get_program(blocks)
    res = run_bass_kernel_spmd(nc, in_maps, core_ids=list(range(NCORES)),
                               trace=trace)
    total = np.zeros((G, G + 1), np.float64)
    for c in range(NCORES):
        total += res.results[c]["partial"].astype(np.float64)
    out = total[:, :G] / np.maximum(total[:, G:G + 1], 1.0)
    return out.astype(np.float32), res


def kernel(**inputs):
    out, _ = _run(inputs, trace=False)
    return out


# revision 23
# speedup vs baseline: 1.6081x; 1.0489x over previous
"""GAT (2-layer, PyG GATConv-style) on 8 Trainium2 NeuronCores.

Strategy (dst-partitioned message passing, memory-optimized):
  - Nodes split into 8 shards of 6250 (padded to 6272 = 49*128 per core).
  - Edges (incl. self-loops) sorted by dst, routed to the dst-owning core,
    grouped into 49 windows of 128 dst nodes, each window padded to fixed
    128-edge blocks (lo/hi split by src half so gather indices fit int16).
  - Layer 1 table rows are 512B: 256 fp8e4m3 h values + 4 bf16 al_src
    scores; the table is AllGathered (25.7MB vs 64MB in f32) and each core
    dma_gathers source rows per edge.
  - Layer 2 table rows are 256B and pack TWO nodes ([64 fp8 h + 1 bf16
    al_src] each); the consumer selects the half by parity, halving the
    AllGather to 6.4MB.
  - al_dst is applied via a transposed one-hot matmul on the tensor engine
    (al_dst of a window's 128 dst nodes stays in SBUF) instead of a
    256B-per-edge gather.
  - Edge-attr loads, al_edge matmuls and one-hot builds are issued between
    the AllGather and the table gathers so they execute during the
    collective.
  - Per-edge: p = exp(leakyrelu(al_src+al_dst+al_edge)) (max-shift skipped;
    logits are O(1)); messages scaled by p on DVE (scalar_tensor_tensor,
    all-SBUF 2x mode) and scatter-added per dst window via bf16 one-hot
    matmuls; softmax denominator rides as extra matmul columns.
  - LayerNorm rsqrt via Ln+Exp so every activation lives in one act table.
  - Graph mean-pool via batch-one-hot matmul; host sums 8 partial [64,65]
    outputs and divides.

Host does only index bookkeeping and small-weight folding; all O(N*F) /
O(E*F) floating-point math runs on device.
"""

import sys

for _p in ("/opt/trn_rl_repo",):
    if _p not in sys.path:
        sys.path.insert(0, _p)

from contextlib import ExitStack

import ml_dtypes
import numpy as np

import concourse.bass as bass
import concourse.mybir as mybir
import concourse.tile as tile
from concourse import bacc
from concourse.bass_utils import run_bass_kernel_spmd

F32 = mybir.dt.float32
BF16 = mybir.dt.bfloat16
F8 = mybir.dt.float8e4
U8 = mybir.dt.uint8
I16 = mybir.dt.int16
AF = mybir.ActivationFunctionType
OP = mybir.AluOpType
BF = ml_dtypes.bfloat16

NCORES = 8
N, E, FIN, ED = 50000, 400000, 128, 6
H, C1, C2, G = 4, 64, 64, 64
EPS = 1e-5
P = 128
SH = N // NCORES            # 6250 nodes per shard
WPC = (SH + P - 1) // P     # 49 windows per core
PADN = WPC * P              # 6272 padded shard rows
HALF = 4 * PADN             # 25088 rows per half-table (int16-safe)
HB2 = PADN // 2             # 3136 pair rows per core (layer-2 table)
ROW1 = 512                  # L1 table row: 256 fp8 h + 4 bf16 al_src
ROW2 = 256                  # L2 table row bytes: 2x(64 fp8 h + 1 bf16 al)
NEG = -1.0e9
EGRP = 1                    # windows per eaT load


def _wrap16(vals):
    """[..., L] int -> [..., 128, L//16] int16, idx j at (j%16, j//16),
    replicated across the 8 gpsimd core windows."""
    lead = vals.shape[:-1]
    L = vals.shape[-1]
    out = np.zeros(lead + (16, L // 16), np.int16)
    jj = np.arange(L)
    out[..., jj % 16, jj // 16] = vals.astype(np.int16)
    return np.tile(out, lead and (1, 8, 1) or (8, 1))


# ----------------------------------------------------------------- host prep
def _prep(inputs):
    x = np.asarray(inputs["x"], np.float32)
    ei = np.asarray(inputs["edge_index"])
    ea = np.asarray(inputs["edge_attr"], np.float32)
    batch = np.asarray(inputs["batch"])
    W1 = np.asarray(inputs["W1"], np.float32)
    We1 = np.asarray(inputs["We1"], np.float32)
    a_src1 = np.asarray(inputs["a_src1"], np.float32)
    a_dst1 = np.asarray(inputs["a_dst1"], np.float32)
    a_edge1 = np.asarray(inputs["a_edge1"], np.float32)
    b1 = np.asarray(inputs["b1"], np.float32)
    ln1_w = np.asarray(inputs["ln1_w"], np.float32)
    ln1_b = np.asarray(inputs["ln1_b"], np.float32)
    W2 = np.asarray(inputs["W2"], np.float32)
    We2 = np.asarray(inputs["We2"], np.float32)
    a_src2 = np.asarray(inputs["a_src2"], np.float32)
    a_dst2 = np.asarray(inputs["a_dst2"], np.float32)
    a_edge2 = np.asarray(inputs["a_edge2"], np.float32)
    b2 = np.asarray(inputs["b2"], np.float32)
    ln2_w = np.asarray(inputs["ln2_w"], np.float32)
    ln2_b = np.asarray(inputs["ln2_b"], np.float32)

    # edges + self loops, sorted by dst
    loop = np.arange(N, dtype=np.int64)
    src = np.concatenate([ei[0].astype(np.int64), loop])
    dst = np.concatenate([ei[1].astype(np.int64), loop])
    order = np.argsort(dst, kind="stable")
    src, dst = src[order], dst[order]
    ea_mean = ea.mean(0)
    ea_sorted = np.empty((len(src), ED), np.float32)
    is_loop = order >= E
    ea_sorted[~is_loop] = ea[order[~is_loop]]
    ea_sorted[is_loop] = ea_mean

    src_core = src // SH
    src_loc = src % SH
    src_row = src_core * PADN + src_loc          # row in the L1 full table
    is_lo = src_row < HALF
    # layer-2 pair-packed rows: local row j holds nodes j and j+HB2
    src_row2 = src_core * HB2 + src_loc % HB2
    src_par2 = src_loc // HB2

    # per (core, window) lo/hi counts -> global fixed block counts
    core_of = dst // SH
    win_of = (dst - core_of * SH) // P
    gwin = core_of * WPC + win_of
    nlo = np.bincount(gwin[is_lo], minlength=NCORES * WPC)
    nhi = np.bincount(gwin[~is_lo], minlength=NCORES * WPC)
    SLB = int(np.ceil(nlo.max() / P))
    SHB = int(np.ceil(nhi.max() / P))
    nbk = SLB + SHB
    slots = nbk * P

    counts = np.bincount(gwin, minlength=NCORES * WPC)
    starts = np.zeros(NCORES * WPC + 1, np.int64)
    np.cumsum(counts, out=starts[1:])

    # folded attention weight vectors
    u1s = (W1.reshape(FIN, H, C1) * a_src1[None]).sum(-1)
    u1d = (W1.reshape(FIN, H, C1) * a_dst1[None]).sum(-1)
    v1 = (We1.reshape(ED, H, C1) * a_edge1[None]).sum(-1)
    u2s = (W2.reshape(H * C1, 1, C2) * a_src2[None]).sum(-1)
    u2d = (W2.reshape(H * C1, 1, C2) * a_dst2[None]).sum(-1)
    v2 = (We2.reshape(ED, 1, C2) * a_edge2[None]).sum(-1)

    w1cat = np.concatenate([W1, u1s, u1d], 1)                 # [128, 264]
    w2cat = np.concatenate([W2, u2s, u2d], 1)                 # [256, 66]
    v1p = np.concatenate([v1, np.full((1, H), NEG, np.float32)], 0)
    v2p = np.concatenate([v2, np.full((1, 1), NEG, np.float32)], 0)

    rep = lambda v: np.broadcast_to(v[None, :], (P, v.shape[0])).copy()
    iota_row = np.broadcast_to(
        np.arange(P, dtype=np.float32)[None, :], (P, P)).astype(BF).copy()
    iotaP2 = np.repeat(np.arange(P, dtype=np.float32)[:, None], 2,
                       1).astype(BF).copy()
    iota64 = np.broadcast_to(
        np.arange(G, dtype=np.float32)[None, :], (P, G)).copy()

    shared = dict(
        w1cat=np.ascontiguousarray(w1cat, np.float32),
        w2a=np.ascontiguousarray(w2cat[:P]).astype(BF),
        w2b=np.ascontiguousarray(w2cat[P:]).astype(BF),
        v1p=v1p.astype(BF), v2p=v2p.astype(BF),
        b1r=rep(b1), ln1wr=rep(ln1_w), ln1br=rep(ln1_b),
        b2r=rep(b2), ln2wr=rep(ln2_w), ln2br=rep(ln2_b),
        iota_row=iota_row, iotaP2=iotaP2, iota64=iota64,
    )
    triv = (not b1.any() and not ln1_b.any() and bool((ln1_w == 1).all()),
            not b2.any() and not ln2_b.any() and bool((ln2_w == 1).all()))

    in_maps = []
    for c in range(NCORES):
        lo = c * SH
        xT = np.zeros((FIN, PADN), np.float32)
        xT[:, :SH] = x[lo:lo + SH].T
        glo = np.zeros((WPC, SLB * P), np.int64)
        ghi = np.zeros((WPC, SHB * P), np.int64)
        g2 = np.zeros((WPC, slots), np.int64)
        par2 = np.zeros((WPC, P, nbk), np.uint8)
        dcol = np.zeros((WPC, P, nbk), np.float32)
        dcolT = np.zeros((WPC, slots), np.float32)
        eaT = np.zeros((WPC, ED + 1, slots), np.float32)
        eaT[:, ED, :] = 1.0  # pad flag on by default
        bcol = np.full((WPC, P), 999.0, np.float32)
        bcol.reshape(-1)[:SH] = batch[lo:lo + SH].astype(np.float32)

        for w in range(WPC):
            g = c * WPC + w
            s, e = starts[g], starts[g + 1]
            if e == s:
                continue
            sr = src_row[s:e]
            sr2 = src_row2[s:e]
            sp2 = src_par2[s:e]
            ed_ = dst[s:e]
            eaw = ea_sorted[s:e]
            ml = sr < HALF
            for half, msel, base, tab in ((0, ml, 0, glo),
                                          (1, ~ml, SLB, ghi)):
                idxs = np.nonzero(msel)[0]
                n_h = len(idxs)
                if n_h == 0:
                    continue
                jj = np.arange(n_h)
                pp, kk = jj % P, base + jj // P
                slot = kk * P + pp
                tab[w, jj] = sr[idxs] - (HALF if half else 0)
                g2[w, slot] = sr2[idxs]
                par2[w, pp, kk] = sp2[idxs].astype(np.uint8)
                dc = (ed_[idxs] - lo - w * P).astype(np.float32)
                dcol[w, pp, kk] = dc
                dcolT[w, slot] = dc
                eaT[w, :ED, slot] = eaw[idxs]  # adv-index: slot axis first
                eaT[w, ED, slot] = 0.0

        m = dict(shared)
        m.update(
            xT=xT,
            glo=_wrap16(glo), ghi=_wrap16(ghi), g2=_wrap16(g2),
            dcol2=np.repeat(dcol.astype(BF)[..., None], 2, -1), par2=par2,
            dcolT=np.broadcast_to(dcolT[:, None, :].astype(BF),
                                  (WPC, P, slots)).copy(),
            eaT=eaT.astype(BF), bcol=bcol,
        )
        in_maps.append(m)
    return in_maps, (SLB, SHB) + triv


# ------------------------------------------------------------- device program
def _build(blocks):
    SLB, SHB, triv1, triv2 = blocks
    nbk = SLB + SHB
    slots = nbk * P
    nc = bacc.Bacc("TRN2", target_bir_lowering=False, debug=False,
                   num_devices=NCORES)
    rg = [list(range(NCORES))]

    t_in = {}
    for name, shape, dt in [
        ("xT", [FIN, PADN], F32),
        ("w1cat", [FIN, 264], F32),
        ("w2a", [P, C2 + 2], BF16), ("w2b", [P, C2 + 2], BF16),
        ("v1p", [ED + 1, H], BF16), ("v2p", [ED + 1, 1], BF16),
        ("b1r", [P, H * C1], F32), ("ln1wr", [P, H * C1], F32),
        ("ln1br", [P, H * C1], F32),
        ("b2r", [P, C2], F32), ("ln2wr", [P, C2], F32),
        ("ln2br", [P, C2], F32),
        ("iota_row", [P, P], BF16), ("iotaP2", [P, 2], BF16),
        ("iota64", [P, G], F32),
        ("glo", [WPC, P, SLB * P // 16], I16),
        ("ghi", [WPC, P, SHB * P // 16], I16),
        ("g2", [WPC, P, slots // 16], I16),
        ("dcol2", [WPC, P, nbk, 2], BF16),
        ("dcolT", [WPC, P, slots], BF16),
        ("par2", [WPC, P, nbk], U8),
        ("eaT", [WPC, ED + 1, slots], BF16),
        ("bcol", [WPC, P], F32),
    ]:
        t_in[name] = nc.dram_tensor(name, shape, dt, kind="ExternalInput")
    out_partial = nc.dram_tensor("partial", [G, G + 1], F32,
                                 kind="ExternalOutput")

    with tile.TileContext(nc) as tc, ExitStack() as ctx:
        const = ctx.enter_context(tc.tile_pool(name="const", bufs=1))
        work = ctx.enter_context(tc.tile_pool(name="work", bufs=3))
        big = ctx.enter_context(tc.tile_pool(name="big", bufs=1))
        psum = ctx.enter_context(tc.tile_pool(name="psum", bufs=2,
                                              space="PSUM"))
        dram = ctx.enter_context(tc.tile_pool(name="dram", bufs=1,
                                              space="DRAM"))

        # const APs used as activation bias operands
        zero_t = const.tile([P, 1], F32)
        nc.vector.memset(zero_t[:], 0.0)
        nc.const_aps.aps[(F32, 0.0)] = zero_t[:]
        eps_t = const.tile([P, 1], F32)
        nc.vector.memset(eps_t[:], EPS)
        nc.const_aps.aps[(F32, EPS)] = eps_t[:]

        def cload(name, dt=F32):
            src_t = t_in[name]
            t = const.tile(list(src_t.shape), dt, name=f"c_{name}")
            nc.sync.dma_start(t[:], src_t.ap())
            return t

        w1cat_sb = cload("w1cat")
        w2a_sb = cload("w2a", BF16)
        w2b_sb = cload("w2b", BF16)
        v1p_sb = cload("v1p", BF16)
        v2p_sb = cload("v2p", BF16)
        b1_sb = cload("b1r"); ln1w_sb = cload("ln1wr"); ln1b_sb = cload("ln1br")
        b2_sb = cload("b2r"); ln2w_sb = cload("ln2wr"); ln2b_sb = cload("ln2br")
        iota_row_sb = cload("iota_row", BF16)
        iotaP2_sb = cload("iotaP2", BF16)
        iota64_sb = cload("iota64")
        from concourse.masks import make_identity
        ident_sb = const.tile([P, P], F32)
        make_identity(nc, ident_sb[:])

        # small per-window tables, loaded once
        dcol_all = const.tile([P, WPC, nbk, 2], BF16)
        nc.sync.dma_start(dcol_all[:],
                          t_in["dcol2"].ap().rearrange("w p k d -> p w k d"))
        par2_all = const.tile([P, WPC, nbk], U8)
        nc.sync.dma_start(par2_all[:],
                          t_in["par2"].ap().rearrange("w p k -> p w k"))
        bcol_all = const.tile([P, WPC], F32)
        nc.sync.dma_start(bcol_all[:],
                          t_in["bcol"].ap().rearrange("w p -> p w"))
        ilo_all = big.tile([P, WPC, SLB * P // 16], I16)
        nc.sync.dma_start(ilo_all[:],
                          t_in["glo"].ap().rearrange("w p c -> p w c"))
        ihi_all = big.tile([P, WPC, SHB * P // 16], I16)
        nc.sync.dma_start(ihi_all[:],
                          t_in["ghi"].ap().rearrange("w p c -> p w c"))
        ig2_all = big.tile([P, WPC, slots // 16], I16)
        nc.sync.dma_start(ig2_all[:],
                          t_in["g2"].ap().rearrange("w p c -> p w c"))

        # DRAM scratch
        bounce1 = dram.tile([PADN, ROW1], U8)
        table1 = dram.tile([NCORES * PADN, ROW1], U8, addr_space="Shared")
        bounce2 = dram.tile([HB2, ROW2], U8)
        table2 = dram.tile([NCORES * HB2, ROW2], U8, addr_space="Shared")

        stage1 = big.tile([P, 25, 264], U8)
        ald1_all = big.tile([P, WPC, H], BF16)
        z01_all = big.tile([P, WPC, nbk * H], BF16)
        z02e_all = big.tile([P, WPC, nbk], BF16)
        var1_all = big.tile([P, WPC], F32)
        isd1_all = big.tile([P, WPC], F32)
        var2_all = big.tile([P, WPC], F32)
        isd2_all = big.tile([P, WPC], F32)
        accum1 = big.tile([P, WPC, 260], F32)
        stage2 = big.tile([P, WPC, P], U8)
        nc.vector.memset(stage2[:], 0)
        ald2_all = big.tile([P, WPC, 1], BF16)
        z02_all = big.tile([P, WPC, nbk], BF16)
        accum2 = big.tile([P, WPC, C2 + 1], F32)
        h3_sb = big.tile([P, WPC, G + 1], F32)

        # ---------------- phase 0: h_aug = x @ [W1|u1s|u1d] -> stage1
        XG = 2
        for half in range(2):
            w0, w1 = (0, 25) if half == 0 else (25, WPC)
            for w in range(w0, w1):
                if (w - w0) % XG == 0:
                    gn = min(XG, w1 - w)
                    xg = work.tile([FIN, XG * P], F32, tag="xg", bufs=2)
                    nc.sync.dma_start(
                        xg[:, 0:gn * P],
                        t_in["xT"].ap()[:, w * P:(w + gn) * P])
                ps0 = psum.tile([P, 264], F32, tag="mm", bufs=3)
                nc.tensor.matmul(
                    ps0[:], lhsT=xg[:, ((w - w0) % XG) * P:
                                    ((w - w0) % XG + 1) * P],
                    rhs=w1cat_sb[:], start=True, stop=True)
                nc.scalar.activation(stage1[:, w - w0, 0:256].bitcast(F8),
                                     ps0[:, 0:256], AF.Copy)
                nc.scalar.activation(stage1[:, w - w0, 256:264].bitcast(BF16),
                                     ps0[:, 256:260], AF.Copy)
                nc.scalar.activation(ald1_all[:, w, :], ps0[:, 260:264],
                                     AF.Copy)
            nc.sync.dma_start(
                bounce1[w0 * P:w1 * P, 0:264]
                .rearrange("(w p) c -> p w c", p=P),
                stage1[:, 0:w1 - w0])
        nc.gpsimd.collective_compute(
            "AllGather", OP.bypass, replica_groups=rg,
            ins=[bounce1.opt()], outs=[table1.opt()])

        # ---------------- edge prep: al_edge + al_dst logits per window
        def build_ohT(dcT, wi):
            ohT = work.tile([P, slots], BF16, tag="ohT", bufs=2)
            nc.vector.tensor_tensor(
                out=ohT[:].rearrange("p (s d) -> p s d", d=2),
                in0=dcT[:, wi].rearrange("p (s d) -> p s d", d=2),
                in1=iotaP2_sb[:].rearrange("p (o d) -> p o d", o=1)
                    .to_broadcast([P, slots // 2, 2]),
                op=OP.is_equal)
            return ohT

        # L1 edge prep (runs during AG1); also precomputes the al_edge part
        # of layer 2 so eaT is loaded only once.
        for w in range(WPC):
            eat = work.tile([ED + 1, slots], BF16, tag="eat", bufs=2)
            nc.sync.dma_start(eat[:], t_in["eaT"].ap()[w])
            dcT = work.tile([P, 1, slots], BF16, tag="dcT", bufs=2)
            nc.sync.dma_start(dcT[:], t_in["dcolT"].ap()[w:w + 1]
                              .rearrange("w p s -> p w s"))
            ohT = build_ohT(dcT, 0)
            zz = psum.tile([P, nbk * H], F32, tag="zps", bufs=2)
            for k in range(nbk):
                nc.tensor.matmul(
                    zz[:, k * H:(k + 1) * H],
                    lhsT=eat[:, k * P:(k + 1) * P],
                    rhs=v1p_sb[:], start=True, stop=False)
                nc.tensor.matmul(
                    zz[:, k * H:(k + 1) * H],
                    lhsT=ohT[:, k * P:(k + 1) * P],
                    rhs=ald1_all[:, w, :],
                    start=False, stop=True)
            nc.scalar.activation(z01_all[:, w, :], zz[:], AF.Copy)
            zz2 = psum.tile([P, nbk], F32, tag="zps", bufs=2)
            for k in range(nbk):
                nc.tensor.matmul(
                    zz2[:, k:k + 1],
                    lhsT=eat[:, k * P:(k + 1) * P],
                    rhs=v2p_sb[:], start=True, stop=True)
            nc.scalar.activation(z02e_all[:, w, :], zz2[:], AF.Copy)

        # ---------------- L1 consume: gather + softmax + scatter
        def build_oh(w):
            oh = work.tile([P, nbk, P], BF16, tag="oh", bufs=3)
            nc.vector.tensor_tensor(
                out=oh[:].rearrange("p k (a d) -> p k a d", d=2),
                in0=iota_row_sb[:].rearrange("p (o a d) -> p o a d", o=1, d=2)
                    .to_broadcast([P, nbk, P // 2, 2]),
                in1=dcol_all[:, w].rearrange("p k (o d) -> p k o d", o=1)
                    .to_broadcast([P, nbk, P // 2, 2]),
                op=OP.is_equal)
            return oh

        for w in range(WPC):
            gb = work.tile([P, nbk, ROW1], U8, tag="gb", bufs=2)
            nc.gpsimd.dma_gather(
                out_ap=gb[:, 0:SLB, :], in_ap=table1[0:HALF, :],
                idxs_ap=ilo_all[:, w], num_idxs=SLB * P,
                num_idxs_reg=SLB * P, elem_size=ROW1, single_packet=False)
            nc.gpsimd.dma_gather(
                out_ap=gb[:, SLB:nbk, :], in_ap=table1[HALF:2 * HALF, :],
                idxs_ap=ihi_all[:, w], num_idxs=SHB * P,
                num_idxs_reg=SHB * P, elem_size=ROW1, single_packet=False)
            oh = build_oh(w)
            z = work.tile([P, nbk, H], BF16, tag="z")
            nc.vector.tensor_tensor(
                out=z[:], in0=gb[:, :, 256:264].bitcast(BF16),
                in1=z01_all[:, w].rearrange("p (k h) -> p k h", h=H),
                op=OP.add)
            nc.vector.scalar_tensor_tensor(
                out=z[:], in0=z[:], scalar=0.2, in1=z[:],
                op0=OP.mult, op1=OP.max)
            pexp = work.tile([P, nbk, H], BF16, tag="pexp")
            nc.scalar.activation(pexp[:], z[:], AF.Exp)
            pexp2 = work.tile([P, nbk, H, 2], BF16, tag="pexp2")
            nc.vector.tensor_copy(
                pexp2[:],
                pexp[:].rearrange("p k (h o) -> p k h o", o=1)
                    .to_broadcast([P, nbk, H, 2]))
            hbf = work.tile([P, nbk, 256], BF16, tag="hbf", bufs=2)
            nc.scalar.activation(hbf[:], gb[:, :, 0:256].bitcast(F8), AF.Copy)
            msg = work.tile([P, nbk, 260], BF16, tag="msg", bufs=2)
            nc.vector.tensor_tensor(
                out=msg[:, :, 0:256]
                    .rearrange("p k (h a d) -> p k h a d", h=H, d=2),
                in0=hbf[:].rearrange("p k (h a d) -> p k h a d", h=H, d=2),
                in1=pexp2[:].rearrange("p k h (o d) -> p k h o d", o=1)
                    .to_broadcast([P, nbk, H, C1 // 2, 2]),
                op=OP.mult)
            nc.vector.tensor_copy(msg[:, :, 256:260], pexp[:])
            sc = psum.tile([P, 260], F32, tag="sc", bufs=2)
            for k in range(nbk):
                nc.tensor.matmul(sc[:], lhsT=oh[:, k, :], rhs=msg[:, k, :],
                                 start=(k == 0), stop=(k == nbk - 1))
            nc.scalar.activation(accum1[:, w, :], sc[:], AF.Copy)

        # ---------------- normalize + relu + LN (batched Ln/Exp)
        def norm_a(acc, nh, feat, y_out, b_sb, var_all, w, triv):
            rec = work.tile([P, nh], F32, tag="rec")
            nc.vector.tensor_scalar_add(rec[:], acc[:, feat:feat + nh], 1e-16)
            nc.vector.reciprocal(rec[:], rec[:])
            nc.vector.scalar_tensor_tensor(
                out=y_out.rearrange("p (h c) -> p h c", h=nh),
                in0=acc[:, 0:feat].rearrange("p (h c) -> p h c", h=nh),
                scalar=1.0,
                in1=rec[:].rearrange("p (h o) -> p h o", o=1)
                    .to_broadcast([P, nh, feat // nh]),
                op0=OP.mult, op1=OP.mult)
            if not triv:
                nc.vector.scalar_tensor_tensor(
                    out=y_out, in0=y_out, scalar=0.0, in1=b_sb[:, 0:feat],
                    op0=OP.add, op1=OP.add)
            nc.scalar.activation(y_out, y_out, AF.Relu)
            mu = work.tile([P, 1], F32, tag="mu")
            nc.vector.tensor_reduce(mu[:], y_out, axis=mybir.AxisListType.X,
                                    op=OP.add)
            mus = work.tile([P, 1], F32, tag="mus")
            nc.scalar.activation(mus[:], mu[:], AF.Copy, scale=1.0 / feat)
            nc.vector.tensor_scalar_sub(y_out, y_out, mus[:, 0:1])
            sq = work.tile([P, feat], F32, tag="sq", bufs=2)
            nc.scalar.activation(sq[:], y_out, AF.Square,
                                 accum_out=var_all[:, w:w + 1])

        def norm_b(y_out, isd_all, w, lnw_sb, lnb_sb, feat, triv):
            nc.vector.tensor_scalar_mul(y_out, y_out, isd_all[:, w:w + 1])
            if not triv:
                nc.vector.scalar_tensor_tensor(
                    out=y_out, in0=y_out, scalar=1.0, in1=lnw_sb[:, 0:feat],
                    op0=OP.mult, op1=OP.mult)
                nc.vector.scalar_tensor_tensor(
                    out=y_out, in0=y_out, scalar=0.0, in1=lnb_sb[:, 0:feat],
                    op0=OP.add, op1=OP.add)

        for w in range(WPC):
            norm_a(accum1[:, w], H, H * C1, accum1[:, w, 0:H * C1], b1_sb,
                   var1_all, w, triv1)
        nc.scalar.activation(isd1_all[:], var1_all[:], AF.Ln, bias=EPS,
                             scale=1.0 / (H * C1))
        nc.scalar.activation(isd1_all[:], isd1_all[:], AF.Exp, scale=-0.5)
        for w in range(WPC):
            norm_b(accum1[:, w, 0:H * C1], isd1_all, w, ln1w_sb, ln1b_sb,
                   H * C1, triv1)

        # ---------------- layer-2 prep: [h2 | als2 | ald2] = h @ w2cat
        for w in range(WPC):
            ps2 = psum.tile([P, C2 + 2], F32, tag="mm", bufs=3)
            for fb in range(2):
                tp = psum.tile([P, P], F32, tag="mm", bufs=3)
                nc.tensor.transpose(tp[:], accum1[:, w, fb * P:(fb + 1) * P],
                                    ident_sb[:])
                tsb = work.tile([P, P], BF16, tag="tsb")
                nc.scalar.activation(tsb[:], tp[:], AF.Copy)
                nc.tensor.matmul(ps2[:], lhsT=tsb[:],
                                 rhs=(w2a_sb[:] if fb == 0 else w2b_sb[:]),
                                 start=(fb == 0), stop=(fb == 1))
            nc.scalar.activation(stage2[:, w, 0:C2].bitcast(F8), ps2[:, 0:C2],
                                 AF.Copy)
            nc.scalar.activation(stage2[:, w, C2:C2 + 2].bitcast(BF16),
                                 ps2[:, C2:C2 + 1], AF.Copy)
            nc.scalar.activation(ald2_all[:, w, :], ps2[:, C2 + 1:C2 + 2],
                                 AF.Copy)
        # pair-packed bounce2 writes: node i -> row i%HB2, col-half i//HB2
        WLO = HB2 // P  # 24 full lo windows, then window 24 straddles
        nc.sync.dma_start(
            bounce2[0:WLO * P, 0:P].rearrange("(w p) c -> p w c", p=P),
            stage2[:, 0:WLO, :])
        nc.sync.dma_start(bounce2[WLO * P:HB2, 0:P],
                          stage2[0:P // 2, WLO, :])
        nc.sync.dma_start(bounce2[0:P // 2, P:ROW2],
                          stage2[P // 2:P, WLO, :])
        nc.sync.dma_start(
            bounce2[P // 2:HB2, P:ROW2]
            .rearrange("(w p) c -> p w c", p=P),
            stage2[:, WLO + 1:WPC, :])
        nc.gpsimd.collective_compute(
            "AllGather", OP.bypass, replica_groups=rg,
            ins=[bounce2.opt()], outs=[table2.opt()])

        for w in range(WPC):
            dcT = work.tile([P, 1, slots], BF16, tag="dcT", bufs=2)
            nc.sync.dma_start(dcT[:], t_in["dcolT"].ap()[w:w + 1]
                              .rearrange("w p s -> p w s"))
            ohT = build_ohT(dcT, 0)
            zz = psum.tile([P, nbk], F32, tag="zps", bufs=2)
            for k in range(nbk):
                nc.tensor.matmul(
                    zz[:, k:k + 1], lhsT=ohT[:, k * P:(k + 1) * P],
                    rhs=ald2_all[:, w, :], start=True, stop=True)
            za = work.tile([P, nbk], BF16, tag="za")
            nc.scalar.activation(za[:], zz[:], AF.Copy)
            nc.vector.tensor_tensor(z02_all[:, w], za[:], z02e_all[:, w],
                                    OP.add)

        tc.no_sync_barrier()
        # ---------------- L2 consume
        for w in range(WPC):
            gb2 = work.tile([P, nbk, ROW2], U8, tag="gb2", bufs=2)
            nc.gpsimd.dma_gather(
                out_ap=gb2[:], in_ap=table2[0:NCORES * HB2, :],
                idxs_ap=ig2_all[:, w], num_idxs=slots,
                num_idxs_reg=slots, elem_size=ROW2, single_packet=False)
            gb2_bf = gb2[:].bitcast(BF16)  # [P, nbk, 128]
            # parity-select the pair half in place into the lo half
            nc.vector.copy_predicated(
                gb2_bf[:, :, 0:33],
                par2_all[:, w].rearrange("p (k o) -> p k o", o=1)
                    .to_broadcast([P, nbk, 33]),
                gb2_bf[:, :, 64:97])
            sel = gb2_bf
            oh = build_oh(w)
            z = work.tile([P, nbk, 1], BF16, tag="z2")
            nc.vector.tensor_tensor(
                out=z[:], in0=sel[:, :, 32:33],
                in1=z02_all[:, w].rearrange("p (k o) -> p k o", o=1),
                op=OP.add)
            nc.vector.scalar_tensor_tensor(
                out=z[:], in0=z[:], scalar=0.2, in1=z[:],
                op0=OP.mult, op1=OP.max)
            pexp = work.tile([P, nbk, 1], BF16, tag="pexpb")
            nc.scalar.activation(pexp[:], z[:], AF.Exp)
            pexp2 = work.tile([P, nbk, 2], BF16, tag="pexpb2")
            nc.vector.tensor_copy(
                pexp2[:], pexp[:].to_broadcast([P, nbk, 2]))
            hbf2 = work.tile([P, nbk, C2], BF16, tag="hbf2", bufs=2)
            nc.scalar.activation(hbf2[:], sel[:, :, 0:32].bitcast(F8),
                                 AF.Copy)
            msg = work.tile([P, nbk, C2 + 1], BF16, tag="msg2", bufs=2)
            nc.vector.tensor_tensor(
                out=msg[:, :, 0:C2].rearrange("p k (a d) -> p k a d", d=2),
                in0=hbf2[:].rearrange("p k (a d) -> p k a d", d=2),
                in1=pexp2[:].rearrange("p k (o d) -> p k o d", o=1)
                    .to_broadcast([P, nbk, C2 // 2, 2]),
                op=OP.mult)
            nc.vector.tensor_copy(msg[:, :, C2:C2 + 1], pexp[:])
            sc = psum.tile([P, C2 + 1], F32, tag="sc", bufs=2)
            for k in range(nbk):
                nc.tensor.matmul(sc[:], lhsT=oh[:, k, :], rhs=msg[:, k, :],
                                 start=(k == 0), stop=(k == nbk - 1))
            nc.scalar.activation(accum2[:, w, :], sc[:], AF.Copy)

        nc.vector.memset(h3_sb[:], 1.0)
        for w in range(WPC):
            norm_a(accum2[:, w], 1, C2, h3_sb[:, w, 0:C2], b2_sb,
                   var2_all, w, triv2)
        nc.scalar.activation(isd2_all[:], var2_all[:], AF.Ln, bias=EPS,
                             scale=1.0 / C2)
        nc.scalar.activation(isd2_all[:], isd2_all[:], AF.Exp, scale=-0.5)
        for w in range(WPC):
            norm_b(h3_sb[:, w, 0:C2], isd2_all, w, ln2w_sb, ln2b_sb, C2,
                   triv2)

        # ---------------- graph mean-pool partials
        pl = psum.tile([G, G + 1], F32, tag="pl", bufs=1)
        for w in range(WPC):
            bh = work.tile([P, G], F32, tag="bh")
            nc.vector.tensor_scalar(
                out=bh[:], in0=iota64_sb[:], scalar1=bcol_all[:, w:w + 1],
                scalar2=None, op0=OP.is_equal)
            nc.tensor.matmul(pl[:], lhsT=bh[:], rhs=h3_sb[:, w, :],
                             start=(w == 0), stop=(w == WPC - 1))
        plo = work.tile([G, G + 1], F32)
        nc.vector.tensor_copy(plo[:], pl[:])
        nc.sync.dma_start(out_partial.ap(), plo[:])

    nc.compile()
    return nc


_CACHE = {}


def _get_program(blocks):
    if blocks not in _CACHE:
        _CACHE[blocks] = _build(blocks)
    return _CACHE[blocks]


def _run(inputs, trace=False):
    in_maps, blocks = _prep(inputs)
    nc = _get_program(blocks)
    res = run_bass_kernel_spmd(nc, in_maps, core_ids=list(range(NCORES)),
                               trace=trace)
    total = np.zeros((G, G + 1), np.float64)
    for c in range(NCORES):
        total += res.results[c]["partial"].astype(np.float64)
    out = total[:, :G] / np.maximum(total[:, G:G + 1], 1.0)
    return out.astype(np.float32), res


def kernel(**inputs):
    out, _ = _run(inputs, trace=False)
    return out
